# revision 1
# baseline (speedup 1.0000x reference)
"""Trainium2 Bass kernel for a 4-layer dense transformer encoder with BatchNorm.

Model (from reference):
  B=128, S=256, D=512, L=4, V=96, H=8, FF=512, DH=64, eps=1e-3
  x = embed[sequence] + pos
  per layer: MHA -> BN(h+attn) -> FFN(relu) -> BN(h+ffn)   (BN in training mode,
  stats over (batch, seq) per feature)

Sharding: data-parallel over batch across 8 cores (16 batches / core).
BN stats are all-reduced (sum, sumsq per feature = 4KB) across cores.

Device layout: activations are kept feature-major ("transposed"):
  hT[feat, token] with feat on partitions (4 tiles of 128) and 4096 tokens free.
All six projections per layer are then natural PE matmuls
  (lhsT = W[feat_in, feat_out], rhs = hT) and BN per-feature scalars are
per-partition tensor_scalar ops.

Attention per (batch, head): scores psum[q=128, k=256] = qT^T kT (K=DH=64,
row-group packed for even/odd heads); exp on ScalarE with accumulated row sums
(no max-subtraction needed: |scores| <~ 1 by construction); the transpose of P
needed for P@V is a regular matmul with diag(1/rowsum) as the moving operand,
folding the softmax normalization in for free; P^T then feeds
attnT[dh, q] = V-lhsT matmuls (col-group packed head pairs).

Both BN affines are folded into the adjacent matmuls rather than applied as
elementwise passes: a2/c2 go into the QKV weights (scaled in-place on device;
Q-bias corrected by a small W^T c matvec; the K/V corrections are provably
softmax/BN-invariant and dropped) and the residual+BN becomes an extra
diag(a) matmul accumulated into the O-proj / W2 psum with the +c added by the
psum-drain copy. rstd is computed on VectorE only (quake rsqrt + 2 Newton
steps) so ScalarE keeps a single activation table (exp) all run long.

Matmul inputs are bf16 (fp32 PSUM accumulate); the residual stream hT stays
fp32. The embedding gather runs on-device as a one-hot matmul: the host builds
a sparse one-hot (vocab + seq-position rows) and the kernel contracts it with
[embed; pos_encodings]. The final BN2-apply + [feat,tok]->[tok,feat]
transpose is fused into per-tile fp32 matmuls (diag(a2) + rank-1 ones x c2row)
feeding contiguous output DMAs.
"""

import numpy as np
import ml_dtypes

# ---------------------------------------------------------------- constants
B, S, D, L, V, H, FF = 128, 256, 512, 4, 96, 8, 512
DH = D // H
EPS = 1e-3
N_CORES = 8
BL = B // N_CORES          # local batches per core
T = BL * S                 # local tokens per core = 4096
P = 128                    # partitions
NF = D // P                # feature tiles = 4
CH = 512                   # token chunk
NCH = T // CH              # chunks = 8
KV = 3                     # one-hot contraction tiles (384 rows / 128)
NT = B * S                 # global token count for BN stats

_BF16 = ml_dtypes.bfloat16

_cache = {}

# tile-pool buffer counts (tunable)
POOL_CFG = dict(cpool1=2, cpool2=2, ppool=8, dpool=16, tpool=8, spool=8,
                opool=3, fpool=2, psA=2, psS=3, psT=2, psV=1)


def _build_bass(sim=False, boring_final=False, no_matvec=False, sqrt_rstd=False, fold=True):
    """Build the Bass program. sim=True builds a single-core variant with the
    AllReduce replaced by a local DRAM copy (for TimelineSim cost analysis)."""
    import concourse.bacc as bacc
    import concourse.tile as tile
    from concourse import mybir
    from concourse.masks import make_identity

    f32 = mybir.dt.float32
    bf16 = mybir.dt.bfloat16
    Alu = mybir.AluOpType
    Act = mybir.ActivationFunctionType

    nc = bacc.Bacc("TRN2", target_bir_lowering=False, debug=False,
                   num_devices=1 if sim else N_CORES)

    # ------------------------------------------------------------ dram I/O
    onehot_d = nc.dram_tensor("onehot", [P, KV, T], bf16,
                              kind="ExternalInput").ap()
    embt_d = nc.dram_tensor("embt", [P, KV, D], bf16,
                            kind="ExternalInput").ap()
    w_d = {}
    for name in ("wq", "wk", "wv", "wo", "w1", "w2"):
        w_d[name] = nc.dram_tensor(name, [L, P, NF, D], bf16,
                                   kind="ExternalInput").ap()
    vec_d = {}
    for name in ("bq", "b1", "g1", "be1", "g2", "be2"):
        vec_d[name] = nc.dram_tensor(name, [L, P, NF], f32,
                                     kind="ExternalInput").ap()
    out_d = nc.dram_tensor("out", [T, D], f32, kind="ExternalOutput").ap()

    with tile.TileContext(nc) as tc:
        from contextlib import ExitStack
        ctx = ExitStack()
        with ctx:
            const = ctx.enter_context(tc.tile_pool(name="const", bufs=1))
            hpool = ctx.enter_context(tc.tile_pool(name="h", bufs=1))
            wpool = ctx.enter_context(tc.tile_pool(name="w", bufs=2))
            wpool1 = ctx.enter_context(tc.tile_pool(name="w1p", bufs=1))
            bpool = ctx.enter_context(tc.tile_pool(name="bias", bufs=2))
            stat = ctx.enter_context(tc.tile_pool(name="stat", bufs=2))
            dramp = ctx.enter_context(tc.tile_pool(name="dramp", bufs=2,
                                                   space="DRAM"))

            hT = hpool.tile([P, NF, T], f32)

            ident_bf = const.tile([P, P], bf16)
            make_identity(nc, ident_bf)
            ident_f32 = const.tile([P, P], f32)
            make_identity(nc, ident_f32)
            eps_sb = const.tile([P, 1], f32)
            nc.vector.memset(eps_sb, EPS)
            ones_f32 = const.tile([1, P], f32)
            nc.vector.memset(ones_f32, 1.0)
            magic_sb = const.tile([P, NF], mybir.dt.uint32)
            nc.vector.memset(magic_sb, 0x5F3759DF)
            one_u32 = const.tile([P, NF], mybir.dt.uint32)
            nc.vector.memset(one_u32, 1)

            # ------------------------------------------------ embedding
            with tc.tile_pool(name="embp", bufs=1) as epool, \
                 tc.tile_pool(name="embps", bufs=4, space="PSUM") as eps_pool:
                oh = epool.tile([P, KV, T], bf16)
                emb = epool.tile([P, KV, D], bf16)
                nc.sync.dma_start(emb[:], embt_d[:])
                # split the big one-hot load so layer-0 matmuls can start
                # as soon as the first token-chunks land
                for t8 in range(NCH):
                    nc.sync.dma_start(oh[:, :, t8 * CH:(t8 + 1) * CH],
                                      onehot_d[:, :, t8 * CH:(t8 + 1) * CH])
                for f in range(NF):
                    for t8 in range(NCH):
                        ps = eps_pool.tile([P, CH], f32, tag="eps")
                        for kc in range(KV):
                            nc.tensor.matmul(
                                ps, lhsT=emb[:, kc, f * P:(f + 1) * P],
                                rhs=oh[:, kc, t8 * CH:(t8 + 1) * CH],
                                start=(kc == 0), stop=(kc == KV - 1))
                        dst = hT[:, f, t8 * CH:(t8 + 1) * CH]
                        if t8 % 2 == 0:
                            nc.vector.tensor_copy(out=dst, in_=ps)
                        else:
                            nc.scalar.copy(out=dst, in_=ps)

            # ------------------------------------------------ layer pools
            cpool1 = ctx.enter_context(tc.tile_pool(name="chunk1", bufs=POOL_CFG["cpool1"]))
            cpool2 = ctx.enter_context(tc.tile_pool(name="chunk2", bufs=POOL_CFG["cpool2"]))
            ppool = ctx.enter_context(tc.tile_pool(name="attn", bufs=POOL_CFG["ppool"]))
            dpool = ctx.enter_context(tc.tile_pool(name="diag", bufs=POOL_CFG["dpool"]))
            tpool = ctx.enter_context(tc.tile_pool(name="ptb", bufs=POOL_CFG["tpool"]))
            spool = ctx.enter_context(tc.tile_pool(name="small", bufs=POOL_CFG["spool"]))
            opool = ctx.enter_context(tc.tile_pool(name="outp", bufs=POOL_CFG["opool"]))
            fpool = ctx.enter_context(tc.tile_pool(name="fold", bufs=POOL_CFG["fpool"]))
            psA = ctx.enter_context(tc.tile_pool(name="psA", bufs=POOL_CFG["psA"],
                                                 space="PSUM"))
            psS = ctx.enter_context(tc.tile_pool(name="psS", bufs=POOL_CFG["psS"],
                                                 space="PSUM"))
            psT = ctx.enter_context(tc.tile_pool(name="psT", bufs=POOL_CFG["psT"],
                                                 space="PSUM"))
            psV = ctx.enter_context(tc.tile_pool(name="psV", bufs=POOL_CFG["psV"],
                                                 space="PSUM"))

            def bn_allreduce(stats_tile, g_sb, be_sb, tag):
                """stats_tile [P, NF, NCH, 6] -> per-feature affine (a, c):
                bn_out = a * z + c, with global (all-core) stats."""
                mv = stat.tile([P, NF, 2], f32, tag=tag + "mv")
                for f in range(NF):
                    nc.vector.bn_aggr(out=mv[:, f, :], in_=stats_tile[:, f, :, :])
                ss = stat.tile([P, NF, 2], f32, tag=tag + "ss")
                tmp = stat.tile([P, NF], f32, tag=tag + "tmp")
                # local sum = mean * T
                nc.vector.tensor_scalar_mul(ss[:, :, 0], mv[:, :, 0], float(T))
                # local sumsq = (var + mean^2) * T
                nc.vector.tensor_tensor(tmp[:], mv[:, :, 0], mv[:, :, 0],
                                        Alu.mult)
                nc.vector.tensor_tensor(tmp[:], tmp[:], mv[:, :, 1], Alu.add)
                # eps folded in here (pre-barrier, off the critical path):
                # after the allreduce, msq - mu^2 = var + eps directly
                nc.vector.tensor_scalar(out=tmp[:], in0=tmp[:],
                                        scalar1=EPS / N_CORES, scalar2=float(T),
                                        op0=Alu.add, op1=Alu.mult)
                nc.vector.tensor_copy(out=ss[:, :, 1], in_=tmp[:])
                din = dramp.tile([P, NF, 2], f32, tag=tag + "din")
                dout = dramp.tile([P, NF, 2], f32, tag=tag + "dout")
                nc.sync.dma_start(din[:], ss[:])
                if sim:
                    nc.sync.dma_start(dout[:], din[:])
                else:
                    nc.gpsimd.collective_compute(
                        "AllReduce", Alu.add,
                        replica_groups=[list(range(N_CORES))],
                        ins=[din.opt()], outs=[dout.opt()])
                gs = stat.tile([P, NF, 2], f32, tag=tag + "gs")
                nc.sync.dma_start(gs[:], dout[:])
                mu = stat.tile([P, NF], f32, tag=tag + "mu")
                var = stat.tile([P, NF], f32, tag=tag + "var")
                nt = float(T if sim else NT)
                nc.vector.tensor_scalar_mul(mu[:], gs[:, :, 0], 1.0 / nt)
                nc.vector.tensor_scalar_mul(var[:], gs[:, :, 1], 1.0 / nt)
                nc.vector.tensor_tensor(tmp[:], mu[:], mu[:], Alu.mult)
                nc.vector.tensor_tensor(var[:], var[:], tmp[:], Alu.subtract)
                if sqrt_rstd:
                    nc.scalar.activation(out=var[:], in_=var[:],
                                         func=Act.Sqrt, bias=eps_sb[:])
                    nc.vector.reciprocal(var[:], var[:])
                else:
                    # rstd = 1/sqrt(var+eps) via quake rsqrt + 2 Newton steps,
                    # entirely on DVE — keeps ScalarE's table set pinned to
                    # exp (no ACT_TABLE_LOAD churn on the BN critical path).
                    # (eps already folded into the all-reduced sumsq)
                    y = stat.tile([P, NF], f32, tag=tag + "y")
                    yi = y.bitcast(mybir.dt.uint32)
                    nc.vector.tensor_tensor(
                        yi[:], var.bitcast(mybir.dt.uint32)[:], one_u32[:],
                        Alu.logical_shift_right)
                    nc.vector.tensor_tensor(yi[:], magic_sb[:], yi[:],
                                            Alu.subtract)
                    t2 = stat.tile([P, NF], f32, tag=tag + "t2")
                    for _ in range(2):
                        nc.vector.tensor_tensor(t2[:], y[:], y[:], Alu.mult)
                        nc.vector.tensor_tensor(t2[:], t2[:], var[:], Alu.mult)
                        nc.vector.tensor_scalar(out=t2[:], in0=t2[:],
                                                scalar1=-0.5, scalar2=1.5,
                                                op0=Alu.mult, op1=Alu.add)
                        nc.vector.tensor_tensor(y[:], y[:], t2[:], Alu.mult)
                    nc.vector.tensor_copy(out=var[:], in_=y[:])
                a = stat.tile([P, NF], f32, tag=tag + "a")
                c = stat.tile([P, NF], f32, tag=tag + "c")
                nc.vector.tensor_tensor(a[:], var[:], g_sb[:], Alu.mult)
                nc.vector.tensor_tensor(c[:], mu[:], a[:], Alu.mult)
                nc.vector.tensor_tensor(c[:], be_sb[:], c[:], Alu.subtract)
                return a, c

            def matvec_w_c(wtile, c_bf, tag):
                """out[:, f] (SBUF psum copy) = sum_kc wtile[:, kc, f*P:]^T
                @ c_bf[:, kc] — the W^T c bias correction for BN folding."""
                mv_ps = psS.tile([P, 2 * S], f32, tag="sps",
                                 name=f"mvps_{tag}")
                if no_matvec:
                    nc.vector.memset(mv_ps[:, :NF], 0.0)
                    return mv_ps
                for f in range(NF):
                    for kc in range(NF):
                        nc.tensor.matmul(
                            mv_ps[:, f:f + 1],
                            lhsT=wtile[:, kc, f * P:(f + 1) * P],
                            rhs=c_bf[:, kc:kc + 1],
                            start=(kc == 0), stop=(kc == NF - 1))
                return mv_ps

            def build_diag(a, tag, dt=None, ident=None):
                """diagA[:, f, :] = diag(a[:, f]), for residual+BN folding
                via PE matmul."""
                dA = fpool.tile([P, NF, P], dt or bf16, tag="diagA",
                                name=f"diagA_{tag}")
                idt = ident or ident_bf
                nc.vector.tensor_tensor(
                    dA[:], idt[:, None, :].to_broadcast((P, NF, P)),
                    a[:, :, None].to_broadcast((P, NF, P)), Alu.mult)
                return dA

            a_pend, c_pend = None, None

            for l in range(L):
                w = {}
                for name in ("wq", "wk", "wv", "wo", "w1", "w2"):
                    pool = wpool if name in ("wq", "wk", "wv") else wpool1
                    w[name] = pool.tile([P, NF, D], bf16, tag=name, name=f"{name}_l{l}")
                    nc.sync.dma_start(w[name][:], w_d[name][l])
                vec = {}
                for name in ("bq", "b1", "g1", "be1", "g2", "be2"):
                    vec[name] = bpool.tile([P, NF], f32, tag=name, name=f"{name}_l{l}")
                    nc.sync.dma_start(vec[name][:], vec_d[name][l])

                # ---- fold previous BN2 (a_pend, c_pend) into QKV weights:
                #   Wx^T(a.z + c) = (a.Wx)^T z + Wx^T c
                # The Wk^T c / Wv^T c terms are dropped: a constant per-row
                # shift of K is softmax-invariant, and a constant shift of V
                # becomes a per-feature constant after attention which the
                # next BN removes. Only Q keeps its correction.
                if a_pend is not None and fold:
                    cbf = stat.tile([P, NF], bf16, tag="cbf2",
                                    name=f"cbf2_{l}")
                    nc.vector.tensor_copy(out=cbf[:], in_=c_pend[:])
                    mv_ps = matvec_w_c(w["wq"], cbf, f"q{l}")
                    bqtot = stat.tile([P, NF], f32, tag="bqtot",
                                      name=f"bqtot_{l}")
                    # bqtot = bq/8 (host pre-scaled) + (Wq^T c)/8
                    nc.vector.tensor_scalar(
                        out=bqtot[:], in0=mv_ps[:, :NF], scalar1=0.125,
                        scalar2=None, op0=Alu.mult)
                    nc.vector.tensor_tensor(bqtot[:], bqtot[:], vec["bq"],
                                            Alu.add)
                    for name in ("wq", "wk", "wv"):
                        for kc in range(NF):
                            nc.vector.tensor_scalar_mul(
                                w[name][:, kc, :], w[name][:, kc, :],
                                a_pend[:, kc:kc + 1])
                    diagA2 = build_diag(a_pend, f"a2_{l}")
                else:
                    bqtot = vec["bq"]
                    diagA2 = None
                c2c = c_pend

                # ======================= phase A: attention =======================
                stats1 = stat.tile([P, NF, NCH, 6], f32, tag="st1")
                for c in range(NCH):
                    tsl = slice(c * CH, (c + 1) * CH)
                    # bf16 input activations (plain cast — BN folded into W)
                    hbf = cpool2.tile([P, NF, CH], bf16, tag="hbf")
                    for f in range(NF):
                        if fold or a_pend is None:
                            nc.gpsimd.tensor_copy(out=hbf[:, f, :],
                                                  in_=hT[:, f, tsl])
                        else:
                            nc.gpsimd.tensor_scalar(
                                out=hbf[:, f, :], in0=hT[:, f, tsl],
                                scalar1=a_pend[:, f:f + 1],
                                scalar2=c_pend[:, f:f + 1],
                                op0=Alu.mult, op1=Alu.add)
                    if not fold and a_pend is not None:
                        for f in range(NF):
                            nc.vector.tensor_scalar(
                                out=hT[:, f, tsl], in0=hT[:, f, tsl],
                                scalar1=a_pend[:, f:f + 1],
                                scalar2=c_pend[:, f:f + 1],
                                op0=Alu.mult, op1=Alu.add)

                    # --- Q, K (transposed), V (token-major) projections
                    qT = cpool1.tile([P, NF, CH], bf16, tag="qT")
                    kT = cpool1.tile([P, NF, CH], bf16, tag="kT")
                    vU = cpool1.tile([P, NF, CH], bf16, tag="vU")
                    for f in range(NF):
                        ps = psA.tile([P, CH], f32, tag="psA")
                        for kc in range(NF):
                            nc.tensor.matmul(
                                ps, lhsT=w["wq"][:, kc, f * P:(f + 1) * P],
                                rhs=hbf[:, kc, :],
                                start=(kc == 0), stop=(kc == NF - 1))
                        nc.vector.tensor_scalar(
                            out=qT[:, f, :], in0=ps, scalar1=0.125,
                            scalar2=bqtot[:, f:f + 1],
                            op0=Alu.mult, op1=Alu.add)
                        ps = psA.tile([P, CH], f32, tag="psA")
                        for kc in range(NF):
                            nc.tensor.matmul(
                                ps, lhsT=w["wk"][:, kc, f * P:(f + 1) * P],
                                rhs=hbf[:, kc, :],
                                start=(kc == 0), stop=(kc == NF - 1))
                        nc.vector.tensor_copy(out=kT[:, f, :], in_=ps)
                    for ts in range(4):
                        ps = psA.tile([P, CH], f32, tag="psA")
                        for kc in range(NF):
                            nc.tensor.matmul(
                                ps, lhsT=hbf[:, kc, ts * P:(ts + 1) * P],
                                rhs=w["wv"][:, kc, :],
                                start=(kc == 0), stop=(kc == NF - 1))
                        nc.vector.tensor_copy(out=vU[:, ts, :], in_=ps)

                    # --- attention for the two batches of this chunk
                    attnT = cpool1.tile([P, NF, CH], bf16, tag="attnT")
                    for bb in range(2):
                        boff = bb * S
                        Ps = {}
                        rrs = [None, None]
                        for qt in range(2):
                            rs = spool.tile([P, H], f32, tag="rs",
                                            name=f"rs_{c}_{bb}_{qt}")
                            rr = spool.tile([P, H], f32, tag="rr",
                                            name=f"rr_{c}_{bb}_{qt}")
                            for h in range(H):
                                if h % 2 == 0:
                                    # per-head-pair P tile: finer lifetime ->
                                    # deeper cross-chunk pipelining
                                    Pp = ppool.tile(
                                        [P, 2, S], bf16, tag="P",
                                        name=f"P_{c}_{bb}_{qt}_{h // 2}")
                                    Ps[qt, h // 2] = Pp
                                po = (h % 2) * DH
                                fi = h // 2
                                sps = psS.tile([P, S], f32, tag="sps")
                                nc.tensor.matmul(
                                    sps,
                                    lhsT=qT[po:po + DH, fi,
                                            boff + qt * P:boff + (qt + 1) * P],
                                    rhs=kT[po:po + DH, fi, boff:boff + S],
                                    start=True, stop=True)
                                # P = exp(scores) (1/8 folded into qT),
                                # rowsum accumulated per head
                                nc.scalar.activation(
                                    out=Ps[qt, h // 2][:, h % 2, :], in_=sps,
                                    func=Act.Exp,
                                    accum_out=rs[:, h:h + 1])
                                if h % 2 == 1:
                                    # reciprocal per head-pair: shortens the
                                    # exp->diag join from 8 exps to 2
                                    nc.vector.reciprocal(rr[:, h - 1:h + 1],
                                                         rs[:, h - 1:h + 1])
                            rrs[qt] = rr
                        # transpose+normalize: PT[k, q] = P[q, k] / rowsum[q]
                        # (regular matmul with diag(1/rowsum) as moving operand)
                        for h in range(H):
                            diag = [None, None]
                            for qt in range(2):
                                dg = dpool.tile([P, P], bf16, tag="dg",
                                                name=f"dg_{c}_{bb}_{h}_{qt}")
                                nc.gpsimd.tensor_scalar_mul(
                                    dg[:], ident_bf[:], rrs[qt][:, h:h + 1])
                                diag[qt] = dg
                            ptb = tpool.tile([P, 2, S], bf16, tag="ptb")
                            pt_ps = psT.tile([P, 2 * S], f32, tag="ptps")
                            for kc in range(2):
                                for qt in range(2):
                                    nc.tensor.matmul(
                                        pt_ps[:, kc * S + qt * P:
                                              kc * S + (qt + 1) * P],
                                        lhsT=Ps[qt, h // 2][:, h % 2,
                                                            kc * P:(kc + 1) * P],
                                        rhs=diag[qt][:],
                                        start=True, stop=True)
                            nc.vector.tensor_copy(out=ptb[:], in_=pt_ps)
                            if h % 4 == 0:
                                avl = psV.tile([P, 2 * S], f32, tag="avps")
                            hh = h % 2
                            jj = (h % 4) // 2
                            for kc in range(2):
                                nc.tensor.matmul(
                                    avl[hh * DH:(hh + 1) * DH,
                                        jj * S:(jj + 1) * S],
                                    lhsT=vU[:, 2 * bb + kc,
                                            h * DH:(h + 1) * DH],
                                    rhs=ptb[:, kc, :],
                                    start=(kc == 0), stop=(kc == 1))
                            if h % 4 == 3:
                                j2 = h // 4  # f-tile pair index (0 or 1)
                                dst = attnT[:, 2 * j2:2 * j2 + 2,
                                            boff:boff + S]
                                nc.vector.tensor_copy(out=dst, in_=avl)
                    # --- O-projection + residual (+ folded BN2 of prev layer)
                    # psum = Wo^T attnT [+ diag(a2) @ z_prev]; copy adds c2.
                    for f in range(NF):
                        ps = psA.tile([P, CH], f32, tag="psA")
                        for kc in range(NF):
                            nc.tensor.matmul(
                                ps, lhsT=w["wo"][:, kc, f * P:(f + 1) * P],
                                rhs=attnT[:, kc, :],
                                start=(kc == 0), stop=False)
                        if not fold:
                            nc.tensor.matmul(
                                ps, lhsT=ident_bf[:], rhs=hbf[:, f, :],
                                start=False, stop=True)
                            nc.vector.tensor_copy(out=hT[:, f, tsl], in_=ps)
                        elif diagA2 is not None:
                            nc.tensor.matmul(
                                ps, lhsT=diagA2[:, f, :],
                                rhs=hbf[:, f, :],
                                start=False, stop=True)
                            nc.scalar.activation(
                                out=hT[:, f, tsl], in_=ps, func=Act.Identity,
                                bias=c2c[:, f:f + 1])
                        else:
                            nc.tensor.matmul(
                                ps, lhsT=ident_bf[:], rhs=hbf[:, f, :],
                                start=False, stop=True)
                            nc.scalar.copy(out=hT[:, f, tsl], in_=ps)
                        nc.vector.bn_stats(out=stats1[:, f, c, :],
                                           in_=hT[:, f, tsl])

                a1, c1 = bn_allreduce(stats1, vec["g1"], vec["be1"], "bn1")

                if fold:
                    # ---- fold BN1 into W1:  relu(W1^T(a1.z+c1) + b1)
                    #    = relu((a1.W1)^T z + (b1 + W1^T c1))
                    c1bf = stat.tile([P, NF], bf16, tag="cbf1",
                                     name=f"cbf1_{l}")
                    nc.vector.tensor_copy(out=c1bf[:], in_=c1[:])
                    mv1_ps = matvec_w_c(w["w1"], c1bf, f"w1_{l}")
                    b1tot = stat.tile([P, NF], f32, tag="b1tot",
                                      name=f"b1tot_{l}")
                    nc.vector.tensor_tensor(b1tot[:], mv1_ps[:, :NF],
                                            vec["b1"], Alu.add)
                    for kc in range(NF):
                        nc.vector.tensor_scalar_mul(
                            w["w1"][:, kc, :], w["w1"][:, kc, :],
                            a1[:, kc:kc + 1])
                    diagA1 = build_diag(a1, f"a1_{l}")
                else:
                    b1tot = vec["b1"]
                    diagA1 = None

                # ======================= phase B: FFN =======================
                stats2 = stat.tile([P, NF, NCH, 6], f32, tag="st2")
                for c in range(NCH):
                    tsl = slice(c * CH, (c + 1) * CH)
                    h1bf = cpool2.tile([P, NF, CH], bf16, tag="hbf",
                                       name=f"h1bf_{l}_{c}")
                    for f in range(NF):
                        if fold:
                            nc.gpsimd.tensor_copy(out=h1bf[:, f, :],
                                                  in_=hT[:, f, tsl])
                        else:
                            nc.gpsimd.tensor_scalar(
                                out=h1bf[:, f, :], in0=hT[:, f, tsl],
                                scalar1=a1[:, f:f + 1],
                                scalar2=c1[:, f:f + 1],
                                op0=Alu.mult, op1=Alu.add)
                    if not fold:
                        for f in range(NF):
                            nc.vector.tensor_scalar(
                                out=hT[:, f, tsl], in0=hT[:, f, tsl],
                                scalar1=a1[:, f:f + 1],
                                scalar2=c1[:, f:f + 1],
                                op0=Alu.mult, op1=Alu.add)
                    ffn = cpool2.tile([P, NF, CH], bf16, tag="ffn")
                    for f in range(NF):
                        ps = psA.tile([P, CH], f32, tag="psA")
                        for kc in range(NF):
                            nc.tensor.matmul(
                                ps, lhsT=w["w1"][:, kc, f * P:(f + 1) * P],
                                rhs=h1bf[:, kc, :],
                                start=(kc == 0), stop=(kc == NF - 1))
                        nc.scalar.activation(out=ffn[:, f, :], in_=ps,
                                             func=Act.Relu,
                                             bias=b1tot[:, f:f + 1])
                    # W2 + folded BN1 residual: psum = W2^T ffn + diag(a1) z1;
                    # copy adds c1 per feature.
                    for f in range(NF):
                        ps = psA.tile([P, CH], f32, tag="psA")
                        for kc in range(NF):
                            nc.tensor.matmul(
                                ps, lhsT=w["w2"][:, kc, f * P:(f + 1) * P],
                                rhs=ffn[:, kc, :],
                                start=(kc == 0), stop=False)
                        if fold:
                            nc.tensor.matmul(
                                ps, lhsT=diagA1[:, f, :], rhs=h1bf[:, f, :],
                                start=False, stop=True)
                            nc.scalar.activation(
                                out=hT[:, f, tsl], in_=ps, func=Act.Identity,
                                bias=c1[:, f:f + 1])
                        else:
                            nc.tensor.matmul(
                                ps, lhsT=ident_bf[:], rhs=h1bf[:, f, :],
                                start=False, stop=True)
                            nc.scalar.copy(out=hT[:, f, tsl], in_=ps)
                        nc.vector.bn_stats(out=stats2[:, f, c, :],
                                           in_=hT[:, f, tsl])

                a_pend, c_pend = bn_allreduce(stats2, vec["g2"], vec["be2"],
                                              "bn2")

            # ========== final: fused BN2-apply + transpose + store ==========
            # out[t, d] = a2[d]*z[d, t] + c2[d], via matmul with diag(a2)
            # plus a rank-1 (ones x c2row) accumulate.
            if boring_final:
                for c in range(NCH):
                    tsl = slice(c * CH, (c + 1) * CH)
                    for f in range(NF):
                        nc.vector.tensor_scalar(
                            out=hT[:, f, tsl], in0=hT[:, f, tsl],
                            scalar1=a_pend[:, f:f + 1],
                            scalar2=c_pend[:, f:f + 1],
                            op0=Alu.mult, op1=Alu.add)
                diagAF = crow = None
            else:
                diagAF = build_diag(a_pend, "final", dt=f32, ident=ident_f32)
                crow_ps = psS.tile([1, 2 * S], f32, tag="sps", name="crow_ps")
                for f in range(NF):
                    nc.tensor.matmul(
                        crow_ps[0:1, f * P:(f + 1) * P],
                        lhsT=c_pend[:, f:f + 1], rhs=ident_f32[:],
                        start=True, stop=True)
                crow = const.tile([1, NF * P], f32)
                nc.vector.tensor_copy(out=crow[:], in_=crow_ps[0:1, :NF * P])
            for tt in range(T // P):
                ops = psA.tile([P, CH], f32, tag="psA")
                for f in range(NF):
                    if boring_final:
                        nc.tensor.matmul(
                            ops[:, f * P:(f + 1) * P],
                            lhsT=hT[:, f, tt * P:(tt + 1) * P],
                            rhs=ident_f32[:],
                            start=True, stop=True)
                        continue
                    nc.tensor.matmul(
                        ops[:, f * P:(f + 1) * P],
                        lhsT=hT[:, f, tt * P:(tt + 1) * P],
                        rhs=diagAF[:, f, :],
                        start=True, stop=False)
                    nc.tensor.matmul(
                        ops[:, f * P:(f + 1) * P],
                        lhsT=ones_f32[:], rhs=crow[0:1, f * P:(f + 1) * P],
                        start=False, stop=True)
                ot = opool.tile([P, CH], f32, tag="ot")
                if tt % 2 == 0:
                    nc.vector.tensor_copy(out=ot[:], in_=ops)
                else:
                    nc.scalar.copy(out=ot[:], in_=ops)
                nc.sync.dma_start(out_d[tt * P:(tt + 1) * P, :], ot[:])

    nc.compile()
    return nc


def _host_prep(inputs):
    """Build per-core in_maps from the full inputs."""
    seq = np.asarray(inputs["sequence"])
    pos = np.asarray(inputs["pos_encodings"], dtype=np.float32)
    emb = np.asarray(inputs["embed"], dtype=np.float32)

    # extended embedding table: rows 0..95 vocab, 96..351 positions, pad to 384
    embt = np.zeros((KV * P, D), np.float32)
    embt[:V] = emb
    embt[V:V + S] = pos
    embt = np.ascontiguousarray(
        embt.reshape(KV, P, D).transpose(1, 0, 2))          # [P, KV, D]

    def wprep(wa):  # [L, D, X] -> [L, P, NF, X] bf16
        wa = np.asarray(wa, dtype=np.float32)
        return np.ascontiguousarray(
            wa.reshape(L, NF, P, wa.shape[-1]).transpose(0, 2, 1, 3)
        ).astype(_BF16)

    def vprep(va):  # [L, D] -> [L, P, NF] f32
        va = np.asarray(va, dtype=np.float32)
        return np.ascontiguousarray(va.reshape(L, NF, P).transpose(0, 2, 1))

    shared = {
        "embt": embt.astype(_BF16),
        "wq": wprep(inputs["Wq"]), "wk": wprep(inputs["Wk"]),
        "wv": wprep(inputs["Wv"]), "wo": wprep(inputs["Wo"]),
        "w1": wprep(inputs["W1"]), "w2": wprep(inputs["W2"]),
        # scores are scaled by 1/sqrt(DH)=1/8 during the Q copy via
        # activation(scale=0.125), which computes in*scale + bias — so the
        # q bias must be pre-scaled here.
        "bq": vprep(np.asarray(inputs["bq"], np.float32) * 0.125),
        "b1": vprep(inputs["b1"]),
        "g1": vprep(inputs["gamma1"]), "be1": vprep(inputs["beta1"]),
        "g2": vprep(inputs["gamma2"]), "be2": vprep(inputs["beta2"]),
    }

    in_maps = []
    for core in range(N_CORES):
        sl = seq[core * BL:(core + 1) * BL].reshape(T).astype(np.int64)
        onehot = np.zeros((KV * P, T), np.float32)
        tok = np.arange(T)
        onehot[sl, tok] = 1.0                       # vocab row
        onehot[V + (tok % S), tok] = 1.0            # position row
        onehot = np.ascontiguousarray(
            onehot.reshape(KV, P, T).transpose(1, 0, 2)).astype(_BF16)
        m = dict(shared)
        m["onehot"] = onehot
        in_maps.append(m)
    return in_maps


def _run(inputs, trace=False):
    from concourse import bass_utils
    if "nc" not in _cache:
        _cache["nc"] = _build_bass()
    nc = _cache["nc"]
    in_maps = _host_prep(inputs)
    res = bass_utils.run_bass_kernel_spmd(
        nc, in_maps, core_ids=list(range(N_CORES)), trace=trace)
    outs = [r["out"].reshape(BL, S, D) for r in res.results]
    full = np.concatenate(outs, axis=0).astype(np.float32)
    return full, res


def kernel(**inputs) -> np.ndarray:
    full, _ = _run(inputs, trace=False)
    return full



# revision 3
# speedup vs baseline: 1.5055x; 1.5055x over previous
"""Trainium2 Bass kernel for a 4-layer dense transformer encoder with BatchNorm.

Model (from reference):
  B=128, S=256, D=512, L=4, V=96, H=8, FF=512, DH=64, eps=1e-3
  x = embed[sequence] + pos
  per layer: MHA -> BN(h+attn) -> FFN(relu) -> BN(h+ffn)   (BN in training mode,
  stats over (batch, seq) per feature)

Sharding: data-parallel over batch across 8 cores (16 batches / core).
BN stats are all-reduced (sum, sumsq per feature = 4KB) across cores.

Device layout: activations are kept feature-major ("transposed"):
  hT[feat, token] with feat on partitions (4 tiles of 128) and 4096 tokens free.
All six projections per layer are then natural PE matmuls
  (lhsT = W[feat_in, feat_out], rhs = hT) and BN per-feature scalars are
per-partition tensor_scalar ops.

Attention per (batch, head): scores psum[q=128, k=256] = qT^T kT (K=DH=64,
row-group packed for even/odd heads); exp on ScalarE with accumulated row sums
(no max-subtraction needed: |scores| <~ 1 by construction); the transpose of P
needed for P@V is a regular matmul with diag(1/rowsum) as the moving operand,
folding the softmax normalization in for free; P^T then feeds
attnT[dh, q] = V-lhsT matmuls (col-group packed head pairs).

Both BN affines are folded into the adjacent matmuls rather than applied as
elementwise passes: a2/c2 go into the QKV weights (scaled in-place on device;
Q-bias corrected by a small W^T c matvec; the K/V corrections are provably
softmax/BN-invariant and dropped) and the residual+BN becomes an extra
diag(a) matmul accumulated into the O-proj / W2 psum with the +c added by the
psum-drain copy. rstd is computed on VectorE only (quake rsqrt + 2 Newton
steps) so ScalarE keeps a single activation table (exp) all run long.

Matmul inputs are bf16 (fp32 PSUM accumulate); the residual stream hT stays
fp32. The embedding gather runs on-device as a one-hot matmul: the host builds
a sparse one-hot (vocab + seq-position rows) and the kernel contracts it with
[embed; pos_encodings]. The final BN2-apply + [feat,tok]->[tok,feat]
transpose is fused into per-tile fp32 matmuls (diag(a2) + rank-1 ones x c2row)
feeding contiguous output DMAs.
"""

import numpy as np
import ml_dtypes

# ---------------------------------------------------------------- constants
B, S, D, L, V, H, FF = 128, 256, 512, 4, 96, 8, 512
DH = D // H
EPS = 1e-3
N_CORES = 8
BL = B // N_CORES          # local batches per core
T = BL * S                 # local tokens per core = 4096
P = 128                    # partitions
NF = D // P                # feature tiles = 4
CH = 512                   # token chunk
NCH = T // CH              # chunks = 8
KV = 3                     # one-hot contraction tiles (384 rows / 128)
NT = B * S                 # global token count for BN stats

_BF16 = ml_dtypes.bfloat16

_cache = {}

# tile-pool buffer counts (tunable)
POOL_CFG = dict(cpool1=2, cpool2=2, ppool=8, dpool=16, tpool=8, spool=8,
                opool=3, fpool=2, psA=2, psS=3, psT=2, psV=1)


def _build_bass(sim=False, boring_final=False, no_matvec=False, sqrt_rstd=False, fold=True,
                no_collective=False):
    """Build the Bass program. sim=True builds a single-core variant with the
    AllReduce replaced by a local DRAM copy (for TimelineSim cost analysis)."""
    import concourse.bacc as bacc
    import concourse.tile as tile
    from concourse import mybir
    from concourse.masks import make_identity

    f32 = mybir.dt.float32
    bf16 = mybir.dt.bfloat16
    Alu = mybir.AluOpType
    Act = mybir.ActivationFunctionType

    nc = bacc.Bacc("TRN2", target_bir_lowering=False, debug=False,
                   num_devices=1 if sim else N_CORES)

    # ------------------------------------------------------------ dram I/O
    onehot_d = nc.dram_tensor("onehot", [P, KV, T], bf16,
                              kind="ExternalInput").ap()
    embt_d = nc.dram_tensor("embt", [P, KV, D], bf16,
                            kind="ExternalInput").ap()
    w_d = {}
    for name in ("wq", "wk", "wv", "wo", "w1", "w2"):
        w_d[name] = nc.dram_tensor(name, [L, P, NF, D], bf16,
                                   kind="ExternalInput").ap()
    vec_d = {}
    for name in ("bq", "b1", "g1", "be1", "g2", "be2"):
        vec_d[name] = nc.dram_tensor(name, [L, P, NF], f32,
                                     kind="ExternalInput").ap()
    out_d = nc.dram_tensor("out", [T, D], f32, kind="ExternalOutput").ap()

    with tile.TileContext(nc) as tc:
        from contextlib import ExitStack
        ctx = ExitStack()
        with ctx:
            const = ctx.enter_context(tc.tile_pool(name="const", bufs=1))
            hpool = ctx.enter_context(tc.tile_pool(name="h", bufs=1))
            wpool = ctx.enter_context(tc.tile_pool(name="w", bufs=2))
            wpool1 = ctx.enter_context(tc.tile_pool(name="w1p", bufs=1))
            bpool = ctx.enter_context(tc.tile_pool(name="bias", bufs=2))
            stat = ctx.enter_context(tc.tile_pool(name="stat", bufs=2))
            dramp = ctx.enter_context(tc.tile_pool(name="dramp", bufs=2,
                                                   space="DRAM"))

            hT = hpool.tile([P, NF, T], f32)

            ident_bf = const.tile([P, P], bf16)
            make_identity(nc, ident_bf)
            ident_f32 = const.tile([P, P], f32)
            make_identity(nc, ident_f32)
            eps_sb = const.tile([P, 1], f32)
            nc.vector.memset(eps_sb, EPS)
            ones_f32 = const.tile([1, P], f32)
            nc.vector.memset(ones_f32, 1.0)
            magic_sb = const.tile([P, NF], mybir.dt.uint32)
            nc.vector.memset(magic_sb, 0x5F3759DF)
            one_u32 = const.tile([P, NF], mybir.dt.uint32)
            nc.vector.memset(one_u32, 1)

            # ------------------------------------------------ embedding
            with tc.tile_pool(name="embp", bufs=1) as epool, \
                 tc.tile_pool(name="embps", bufs=4, space="PSUM") as eps_pool:
                oh = epool.tile([P, KV, T], bf16)
                emb = epool.tile([P, KV, D], bf16)
                nc.sync.dma_start(emb[:], embt_d[:])
                # split the big one-hot load so layer-0 matmuls can start
                # as soon as the first token-chunks land
                for t8 in range(NCH):
                    nc.sync.dma_start(oh[:, :, t8 * CH:(t8 + 1) * CH],
                                      onehot_d[:, :, t8 * CH:(t8 + 1) * CH])
                for f in range(NF):
                    for t8 in range(NCH):
                        ps = eps_pool.tile([P, CH], f32, tag="eps")
                        for kc in range(KV):
                            nc.tensor.matmul(
                                ps, lhsT=emb[:, kc, f * P:(f + 1) * P],
                                rhs=oh[:, kc, t8 * CH:(t8 + 1) * CH],
                                start=(kc == 0), stop=(kc == KV - 1))
                        dst = hT[:, f, t8 * CH:(t8 + 1) * CH]
                        if t8 % 2 == 0:
                            nc.vector.tensor_copy(out=dst, in_=ps)
                        else:
                            nc.scalar.copy(out=dst, in_=ps)

            # ------------------------------------------------ layer pools
            cpool1 = ctx.enter_context(tc.tile_pool(name="chunk1", bufs=POOL_CFG["cpool1"]))
            cpool2 = ctx.enter_context(tc.tile_pool(name="chunk2", bufs=POOL_CFG["cpool2"]))
            ppool = ctx.enter_context(tc.tile_pool(name="attn", bufs=POOL_CFG["ppool"]))
            dpool = ctx.enter_context(tc.tile_pool(name="diag", bufs=POOL_CFG["dpool"]))
            tpool = ctx.enter_context(tc.tile_pool(name="ptb", bufs=POOL_CFG["tpool"]))
            spool = ctx.enter_context(tc.tile_pool(name="small", bufs=POOL_CFG["spool"]))
            opool = ctx.enter_context(tc.tile_pool(name="outp", bufs=POOL_CFG["opool"]))
            fpool = ctx.enter_context(tc.tile_pool(name="fold", bufs=POOL_CFG["fpool"]))
            psA = ctx.enter_context(tc.tile_pool(name="psA", bufs=POOL_CFG["psA"],
                                                 space="PSUM"))
            psS = ctx.enter_context(tc.tile_pool(name="psS", bufs=POOL_CFG["psS"],
                                                 space="PSUM"))
            psT = ctx.enter_context(tc.tile_pool(name="psT", bufs=POOL_CFG["psT"],
                                                 space="PSUM"))
            psV = ctx.enter_context(tc.tile_pool(name="psV", bufs=POOL_CFG["psV"],
                                                 space="PSUM"))

            def bn_allreduce(stats_tile, g_sb, be_sb, tag):
                """stats_tile [P, NF, NCH, 6] -> per-feature affine (a, c):
                bn_out = a * z + c, with global (all-core) stats."""
                mv = stat.tile([P, NF, 2], f32, tag=tag + "mv")
                for f in range(NF):
                    nc.vector.bn_aggr(out=mv[:, f, :], in_=stats_tile[:, f, :, :])
                ss = stat.tile([P, NF, 2], f32, tag=tag + "ss")
                tmp = stat.tile([P, NF], f32, tag=tag + "tmp")
                # local sum = mean * T
                nc.vector.tensor_scalar_mul(ss[:, :, 0], mv[:, :, 0], float(T))
                # local sumsq = (var + mean^2) * T
                nc.vector.tensor_tensor(tmp[:], mv[:, :, 0], mv[:, :, 0],
                                        Alu.mult)
                nc.vector.tensor_tensor(tmp[:], tmp[:], mv[:, :, 1], Alu.add)
                # eps folded in here (pre-barrier, off the critical path):
                # after the allreduce, msq - mu^2 = var + eps directly
                nc.vector.tensor_scalar(out=tmp[:], in0=tmp[:],
                                        scalar1=EPS / N_CORES, scalar2=float(T),
                                        op0=Alu.add, op1=Alu.mult)
                nc.vector.tensor_copy(out=ss[:, :, 1], in_=tmp[:])
                din = dramp.tile([P, NF, 2], f32, tag=tag + "din")
                dout = dramp.tile([P, NF, 2], f32, tag=tag + "dout")
                nc.sync.dma_start(din[:], ss[:])
                if sim or no_collective:
                    nc.sync.dma_start(dout[:], din[:])
                else:
                    nc.gpsimd.collective_compute(
                        "AllReduce", Alu.add,
                        replica_groups=[list(range(N_CORES))],
                        ins=[din.opt()], outs=[dout.opt()])
                gs = stat.tile([P, NF, 2], f32, tag=tag + "gs")
                nc.sync.dma_start(gs[:], dout[:])
                mu = stat.tile([P, NF], f32, tag=tag + "mu")
                var = stat.tile([P, NF], f32, tag=tag + "var")
                nt = float(T if sim else NT)
                nc.vector.tensor_scalar_mul(mu[:], gs[:, :, 0], 1.0 / nt)
                nc.vector.tensor_scalar_mul(var[:], gs[:, :, 1], 1.0 / nt)
                nc.vector.tensor_tensor(tmp[:], mu[:], mu[:], Alu.mult)
                nc.vector.tensor_tensor(var[:], var[:], tmp[:], Alu.subtract)
                if sqrt_rstd:
                    nc.scalar.activation(out=var[:], in_=var[:],
                                         func=Act.Sqrt, bias=eps_sb[:])
                    nc.vector.reciprocal(var[:], var[:])
                else:
                    # rstd = 1/sqrt(var+eps) via quake rsqrt + 2 Newton steps,
                    # entirely on DVE — keeps ScalarE's table set pinned to
                    # exp (no ACT_TABLE_LOAD churn on the BN critical path).
                    # (eps already folded into the all-reduced sumsq)
                    y = stat.tile([P, NF], f32, tag=tag + "y")
                    yi = y.bitcast(mybir.dt.uint32)
                    nc.vector.tensor_tensor(
                        yi[:], var.bitcast(mybir.dt.uint32)[:], one_u32[:],
                        Alu.logical_shift_right)
                    nc.vector.tensor_tensor(yi[:], magic_sb[:], yi[:],
                                            Alu.subtract)
                    t2 = stat.tile([P, NF], f32, tag=tag + "t2")
                    for _ in range(2):
                        nc.vector.tensor_tensor(t2[:], y[:], y[:], Alu.mult)
                        nc.vector.tensor_tensor(t2[:], t2[:], var[:], Alu.mult)
                        nc.vector.tensor_scalar(out=t2[:], in0=t2[:],
                                                scalar1=-0.5, scalar2=1.5,
                                                op0=Alu.mult, op1=Alu.add)
                        nc.vector.tensor_tensor(y[:], y[:], t2[:], Alu.mult)
                    nc.vector.tensor_copy(out=var[:], in_=y[:])
                a = stat.tile([P, NF], f32, tag=tag + "a")
                c = stat.tile([P, NF], f32, tag=tag + "c")
                nc.vector.tensor_tensor(a[:], var[:], g_sb[:], Alu.mult)
                nc.vector.tensor_tensor(c[:], mu[:], a[:], Alu.mult)
                nc.vector.tensor_tensor(c[:], be_sb[:], c[:], Alu.subtract)
                return a, c

            def matvec_w_c(wtile, c_bf, tag):
                """out[:, f] (SBUF psum copy) = sum_kc wtile[:, kc, f*P:]^T
                @ c_bf[:, kc] — the W^T c bias correction for BN folding."""
                mv_ps = psS.tile([P, 2 * S], f32, tag="sps",
                                 name=f"mvps_{tag}")
                if no_matvec:
                    nc.vector.memset(mv_ps[:, :NF], 0.0)
                    return mv_ps
                for f in range(NF):
                    for kc in range(NF):
                        nc.tensor.matmul(
                            mv_ps[:, f:f + 1],
                            lhsT=wtile[:, kc, f * P:(f + 1) * P],
                            rhs=c_bf[:, kc:kc + 1],
                            start=(kc == 0), stop=(kc == NF - 1))
                return mv_ps

            def build_diag(a, tag, dt=None, ident=None):
                """diagA[:, f, :] = diag(a[:, f]), for residual+BN folding
                via PE matmul."""
                dA = fpool.tile([P, NF, P], dt or bf16, tag="diagA",
                                name=f"diagA_{tag}")
                idt = ident or ident_bf
                nc.vector.tensor_tensor(
                    dA[:], idt[:, None, :].to_broadcast((P, NF, P)),
                    a[:, :, None].to_broadcast((P, NF, P)), Alu.mult)
                return dA

            a_pend, c_pend = None, None

            for l in range(L):
                w = {}
                for name in ("wq", "wk", "wv", "wo", "w1", "w2"):
                    pool = wpool if name in ("wq", "wk", "wv") else wpool1
                    w[name] = pool.tile([P, NF, D], bf16, tag=name, name=f"{name}_l{l}")
                    nc.sync.dma_start(w[name][:], w_d[name][l])
                vec = {}
                for name in ("bq", "b1", "g1", "be1", "g2", "be2"):
                    vec[name] = bpool.tile([P, NF], f32, tag=name, name=f"{name}_l{l}")
                    nc.sync.dma_start(vec[name][:], vec_d[name][l])

                # ---- fold previous BN2 (a_pend, c_pend) into QKV weights:
                #   Wx^T(a.z + c) = (a.Wx)^T z + Wx^T c
                # The Wk^T c / Wv^T c terms are dropped: a constant per-row
                # shift of K is softmax-invariant, and a constant shift of V
                # becomes a per-feature constant after attention which the
                # next BN removes. Only Q keeps its correction.
                if a_pend is not None and fold:
                    cbf = stat.tile([P, NF], bf16, tag="cbf2",
                                    name=f"cbf2_{l}")
                    nc.vector.tensor_copy(out=cbf[:], in_=c_pend[:])
                    mv_ps = matvec_w_c(w["wq"], cbf, f"q{l}")
                    bqtot = stat.tile([P, NF], f32, tag="bqtot",
                                      name=f"bqtot_{l}")
                    # bqtot = bq/8 (host pre-scaled) + (Wq^T c)/8
                    nc.vector.tensor_scalar(
                        out=bqtot[:], in0=mv_ps[:, :NF], scalar1=0.125,
                        scalar2=None, op0=Alu.mult)
                    nc.vector.tensor_tensor(bqtot[:], bqtot[:], vec["bq"],
                                            Alu.add)
                    for name in ("wq", "wk", "wv"):
                        for kc in range(NF):
                            nc.vector.tensor_scalar_mul(
                                w[name][:, kc, :], w[name][:, kc, :],
                                a_pend[:, kc:kc + 1])
                    diagA2 = build_diag(a_pend, f"a2_{l}")
                else:
                    bqtot = vec["bq"]
                    diagA2 = None
                c2c = c_pend

                # ======================= phase A: attention =======================
                stats1 = stat.tile([P, NF, NCH, 6], f32, tag="st1")
                for c in range(NCH):
                    tsl = slice(c * CH, (c + 1) * CH)
                    # bf16 input activations (plain cast — BN folded into W)
                    hbf = cpool2.tile([P, NF, CH], bf16, tag="hbf")
                    for f in range(NF):
                        if fold or a_pend is None:
                            nc.gpsimd.tensor_copy(out=hbf[:, f, :],
                                                  in_=hT[:, f, tsl])
                        else:
                            nc.gpsimd.tensor_scalar(
                                out=hbf[:, f, :], in0=hT[:, f, tsl],
                                scalar1=a_pend[:, f:f + 1],
                                scalar2=c_pend[:, f:f + 1],
                                op0=Alu.mult, op1=Alu.add)
                    if not fold and a_pend is not None:
                        for f in range(NF):
                            nc.vector.tensor_scalar(
                                out=hT[:, f, tsl], in0=hT[:, f, tsl],
                                scalar1=a_pend[:, f:f + 1],
                                scalar2=c_pend[:, f:f + 1],
                                op0=Alu.mult, op1=Alu.add)

                    # --- Q, K (transposed), V (token-major) projections
                    qT = cpool1.tile([P, NF, CH], bf16, tag="qT")
                    kT = cpool1.tile([P, NF, CH], bf16, tag="kT")
                    vU = cpool1.tile([P, NF, CH], bf16, tag="vU")
                    for f in range(NF):
                        ps = psA.tile([P, CH], f32, tag="psA")
                        for kc in range(NF):
                            nc.tensor.matmul(
                                ps, lhsT=w["wq"][:, kc, f * P:(f + 1) * P],
                                rhs=hbf[:, kc, :],
                                start=(kc == 0), stop=(kc == NF - 1))
                        nc.vector.tensor_scalar(
                            out=qT[:, f, :], in0=ps, scalar1=0.125,
                            scalar2=bqtot[:, f:f + 1],
                            op0=Alu.mult, op1=Alu.add)
                        ps = psA.tile([P, CH], f32, tag="psA")
                        for kc in range(NF):
                            nc.tensor.matmul(
                                ps, lhsT=w["wk"][:, kc, f * P:(f + 1) * P],
                                rhs=hbf[:, kc, :],
                                start=(kc == 0), stop=(kc == NF - 1))
                        nc.vector.tensor_copy(out=kT[:, f, :], in_=ps)
                    for ts in range(4):
                        ps = psA.tile([P, CH], f32, tag="psA")
                        for kc in range(NF):
                            nc.tensor.matmul(
                                ps, lhsT=hbf[:, kc, ts * P:(ts + 1) * P],
                                rhs=w["wv"][:, kc, :],
                                start=(kc == 0), stop=(kc == NF - 1))
                        nc.vector.tensor_copy(out=vU[:, ts, :], in_=ps)

                    # --- attention for the two batches of this chunk
                    attnT = cpool1.tile([P, NF, CH], bf16, tag="attnT")
                    for bb in range(2):
                        boff = bb * S
                        Ps = {}
                        rrs = [None, None]
                        for qt in range(2):
                            rs = spool.tile([P, H], f32, tag="rs",
                                            name=f"rs_{c}_{bb}_{qt}")
                            rr = spool.tile([P, H], f32, tag="rr",
                                            name=f"rr_{c}_{bb}_{qt}")
                            for h in range(H):
                                if h % 2 == 0:
                                    # per-head-pair P tile: finer lifetime ->
                                    # deeper cross-chunk pipelining
                                    Pp = ppool.tile(
                                        [P, 2, S], bf16, tag="P",
                                        name=f"P_{c}_{bb}_{qt}_{h // 2}")
                                    Ps[qt, h // 2] = Pp
                                po = (h % 2) * DH
                                fi = h // 2
                                sps = psS.tile([P, S], f32, tag="sps")
                                nc.tensor.matmul(
                                    sps,
                                    lhsT=qT[po:po + DH, fi,
                                            boff + qt * P:boff + (qt + 1) * P],
                                    rhs=kT[po:po + DH, fi, boff:boff + S],
                                    start=True, stop=True)
                                # P = exp(scores) (1/8 folded into qT),
                                # rowsum accumulated per head
                                nc.scalar.activation(
                                    out=Ps[qt, h // 2][:, h % 2, :], in_=sps,
                                    func=Act.Exp,
                                    accum_out=rs[:, h:h + 1])
                                if h % 2 == 1:
                                    # reciprocal per head-pair: shortens the
                                    # exp->diag join from 8 exps to 2
                                    nc.vector.reciprocal(rr[:, h - 1:h + 1],
                                                         rs[:, h - 1:h + 1])
                            rrs[qt] = rr
                        # transpose+normalize: PT[k, q] = P[q, k] / rowsum[q]
                        # (regular matmul with diag(1/rowsum) as moving operand)
                        for h in range(H):
                            diag = [None, None]
                            for qt in range(2):
                                dg = dpool.tile([P, P], bf16, tag="dg",
                                                name=f"dg_{c}_{bb}_{h}_{qt}")
                                nc.gpsimd.tensor_scalar_mul(
                                    dg[:], ident_bf[:], rrs[qt][:, h:h + 1])
                                diag[qt] = dg
                            ptb = tpool.tile([P, 2, S], bf16, tag="ptb")
                            pt_ps = psT.tile([P, 2 * S], f32, tag="ptps")
                            for kc in range(2):
                                for qt in range(2):
                                    nc.tensor.matmul(
                                        pt_ps[:, kc * S + qt * P:
                                              kc * S + (qt + 1) * P],
                                        lhsT=Ps[qt, h // 2][:, h % 2,
                                                            kc * P:(kc + 1) * P],
                                        rhs=diag[qt][:],
                                        start=True, stop=True)
                            nc.vector.tensor_copy(out=ptb[:], in_=pt_ps)
                            if h % 4 == 0:
                                avl = psV.tile([P, 2 * S], f32, tag="avps")
                            hh = h % 2
                            jj = (h % 4) // 2
                            for kc in range(2):
                                nc.tensor.matmul(
                                    avl[hh * DH:(hh + 1) * DH,
                                        jj * S:(jj + 1) * S],
                                    lhsT=vU[:, 2 * bb + kc,
                                            h * DH:(h + 1) * DH],
                                    rhs=ptb[:, kc, :],
                                    start=(kc == 0), stop=(kc == 1))
                            if h % 4 == 3:
                                j2 = h // 4  # f-tile pair index (0 or 1)
                                dst = attnT[:, 2 * j2:2 * j2 + 2,
                                            boff:boff + S]
                                nc.vector.tensor_copy(out=dst, in_=avl)
                    # --- O-projection + residual (+ folded BN2 of prev layer)
                    # psum = Wo^T attnT [+ diag(a2) @ z_prev]; copy adds c2.
                    for f in range(NF):
                        ps = psA.tile([P, CH], f32, tag="psA")
                        for kc in range(NF):
                            nc.tensor.matmul(
                                ps, lhsT=w["wo"][:, kc, f * P:(f + 1) * P],
                                rhs=attnT[:, kc, :],
                                start=(kc == 0), stop=False)
                        if not fold:
                            nc.tensor.matmul(
                                ps, lhsT=ident_bf[:], rhs=hbf[:, f, :],
                                start=False, stop=True)
                            nc.vector.tensor_copy(out=hT[:, f, tsl], in_=ps)
                        elif diagA2 is not None:
                            nc.tensor.matmul(
                                ps, lhsT=diagA2[:, f, :],
                                rhs=hbf[:, f, :],
                                start=False, stop=True)
                            nc.scalar.activation(
                                out=hT[:, f, tsl], in_=ps, func=Act.Identity,
                                bias=c2c[:, f:f + 1])
                        else:
                            nc.tensor.matmul(
                                ps, lhsT=ident_bf[:], rhs=hbf[:, f, :],
                                start=False, stop=True)
                            nc.scalar.copy(out=hT[:, f, tsl], in_=ps)
                        nc.vector.bn_stats(out=stats1[:, f, c, :],
                                           in_=hT[:, f, tsl])

                a1, c1 = bn_allreduce(stats1, vec["g1"], vec["be1"], "bn1")

                if fold:
                    # ---- fold BN1 into W1:  relu(W1^T(a1.z+c1) + b1)
                    #    = relu((a1.W1)^T z + (b1 + W1^T c1))
                    c1bf = stat.tile([P, NF], bf16, tag="cbf1",
                                     name=f"cbf1_{l}")
                    nc.vector.tensor_copy(out=c1bf[:], in_=c1[:])
                    mv1_ps = matvec_w_c(w["w1"], c1bf, f"w1_{l}")
                    b1tot = stat.tile([P, NF], f32, tag="b1tot",
                                      name=f"b1tot_{l}")
                    nc.vector.tensor_tensor(b1tot[:], mv1_ps[:, :NF],
                                            vec["b1"], Alu.add)
                    for kc in range(NF):
                        nc.vector.tensor_scalar_mul(
                            w["w1"][:, kc, :], w["w1"][:, kc, :],
                            a1[:, kc:kc + 1])
                    diagA1 = build_diag(a1, f"a1_{l}")
                else:
                    b1tot = vec["b1"]
                    diagA1 = None

                # ======================= phase B: FFN =======================
                stats2 = stat.tile([P, NF, NCH, 6], f32, tag="st2")
                for c in range(NCH):
                    tsl = slice(c * CH, (c + 1) * CH)
                    h1bf = cpool2.tile([P, NF, CH], bf16, tag="hbf",
                                       name=f"h1bf_{l}_{c}")
                    for f in range(NF):
                        if fold:
                            nc.gpsimd.tensor_copy(out=h1bf[:, f, :],
                                                  in_=hT[:, f, tsl])
                        else:
                            nc.gpsimd.tensor_scalar(
                                out=h1bf[:, f, :], in0=hT[:, f, tsl],
                                scalar1=a1[:, f:f + 1],
                                scalar2=c1[:, f:f + 1],
                                op0=Alu.mult, op1=Alu.add)
                    if not fold:
                        for f in range(NF):
                            nc.vector.tensor_scalar(
                                out=hT[:, f, tsl], in0=hT[:, f, tsl],
                                scalar1=a1[:, f:f + 1],
                                scalar2=c1[:, f:f + 1],
                                op0=Alu.mult, op1=Alu.add)
                    ffn = cpool2.tile([P, NF, CH], bf16, tag="ffn")
                    for f in range(NF):
                        ps = psA.tile([P, CH], f32, tag="psA")
                        for kc in range(NF):
                            nc.tensor.matmul(
                                ps, lhsT=w["w1"][:, kc, f * P:(f + 1) * P],
                                rhs=h1bf[:, kc, :],
                                start=(kc == 0), stop=(kc == NF - 1))
                        nc.scalar.activation(out=ffn[:, f, :], in_=ps,
                                             func=Act.Relu,
                                             bias=b1tot[:, f:f + 1])
                    # W2 + folded BN1 residual: psum = W2^T ffn + diag(a1) z1;
                    # copy adds c1 per feature.
                    for f in range(NF):
                        ps = psA.tile([P, CH], f32, tag="psA")
                        for kc in range(NF):
                            nc.tensor.matmul(
                                ps, lhsT=w["w2"][:, kc, f * P:(f + 1) * P],
                                rhs=ffn[:, kc, :],
                                start=(kc == 0), stop=False)
                        if fold:
                            nc.tensor.matmul(
                                ps, lhsT=diagA1[:, f, :], rhs=h1bf[:, f, :],
                                start=False, stop=True)
                            nc.scalar.activation(
                                out=hT[:, f, tsl], in_=ps, func=Act.Identity,
                                bias=c1[:, f:f + 1])
                        else:
                            nc.tensor.matmul(
                                ps, lhsT=ident_bf[:], rhs=h1bf[:, f, :],
                                start=False, stop=True)
                            nc.scalar.copy(out=hT[:, f, tsl], in_=ps)
                        nc.vector.bn_stats(out=stats2[:, f, c, :],
                                           in_=hT[:, f, tsl])

                a_pend, c_pend = bn_allreduce(stats2, vec["g2"], vec["be2"],
                                              "bn2")

            # ========== final: fused BN2-apply + transpose + store ==========
            # out[t, d] = a2[d]*z[d, t] + c2[d], via matmul with diag(a2)
            # plus a rank-1 (ones x c2row) accumulate.
            if boring_final:
                for c in range(NCH):
                    tsl = slice(c * CH, (c + 1) * CH)
                    for f in range(NF):
                        nc.vector.tensor_scalar(
                            out=hT[:, f, tsl], in0=hT[:, f, tsl],
                            scalar1=a_pend[:, f:f + 1],
                            scalar2=c_pend[:, f:f + 1],
                            op0=Alu.mult, op1=Alu.add)
                diagAF = crow = None
            else:
                diagAF = build_diag(a_pend, "final", dt=f32, ident=ident_f32)
                crow_ps = psS.tile([1, 2 * S], f32, tag="sps", name="crow_ps")
                for f in range(NF):
                    nc.tensor.matmul(
                        crow_ps[0:1, f * P:(f + 1) * P],
                        lhsT=c_pend[:, f:f + 1], rhs=ident_f32[:],
                        start=True, stop=True)
                crow = const.tile([1, NF * P], f32)
                nc.vector.tensor_copy(out=crow[:], in_=crow_ps[0:1, :NF * P])
            for tt in range(T // P):
                ops = psA.tile([P, CH], f32, tag="psA")
                for f in range(NF):
                    if boring_final:
                        nc.tensor.matmul(
                            ops[:, f * P:(f + 1) * P],
                            lhsT=hT[:, f, tt * P:(tt + 1) * P],
                            rhs=ident_f32[:],
                            start=True, stop=True)
                        continue
                    nc.tensor.matmul(
                        ops[:, f * P:(f + 1) * P],
                        lhsT=hT[:, f, tt * P:(tt + 1) * P],
                        rhs=diagAF[:, f, :],
                        start=True, stop=False)
                    nc.tensor.matmul(
                        ops[:, f * P:(f + 1) * P],
                        lhsT=ones_f32[:], rhs=crow[0:1, f * P:(f + 1) * P],
                        start=False, stop=True)
                ot = opool.tile([P, CH], f32, tag="ot")
                if tt % 2 == 0:
                    nc.vector.tensor_copy(out=ot[:], in_=ops)
                else:
                    nc.scalar.copy(out=ot[:], in_=ops)
                nc.sync.dma_start(out_d[tt * P:(tt + 1) * P, :], ot[:])

    nc.compile()
    return nc


def _host_prep(inputs):
    """Build per-core in_maps from the full inputs."""
    seq = np.asarray(inputs["sequence"])
    pos = np.asarray(inputs["pos_encodings"], dtype=np.float32)
    emb = np.asarray(inputs["embed"], dtype=np.float32)

    # extended embedding table: rows 0..95 vocab, 96..351 positions, pad to 384
    embt = np.zeros((KV * P, D), np.float32)
    embt[:V] = emb
    embt[V:V + S] = pos
    embt = np.ascontiguousarray(
        embt.reshape(KV, P, D).transpose(1, 0, 2))          # [P, KV, D]

    def wprep(wa):  # [L, D, X] -> [L, P, NF, X] bf16
        wa = np.asarray(wa, dtype=np.float32)
        return np.ascontiguousarray(
            wa.reshape(L, NF, P, wa.shape[-1]).transpose(0, 2, 1, 3)
        ).astype(_BF16)

    def vprep(va):  # [L, D] -> [L, P, NF] f32
        va = np.asarray(va, dtype=np.float32)
        return np.ascontiguousarray(va.reshape(L, NF, P).transpose(0, 2, 1))

    shared = {
        "embt": embt.astype(_BF16),
        "wq": wprep(inputs["Wq"]), "wk": wprep(inputs["Wk"]),
        "wv": wprep(inputs["Wv"]), "wo": wprep(inputs["Wo"]),
        "w1": wprep(inputs["W1"]), "w2": wprep(inputs["W2"]),
        # scores are scaled by 1/sqrt(DH)=1/8 during the Q copy via
        # activation(scale=0.125), which computes in*scale + bias — so the
        # q bias must be pre-scaled here.
        "bq": vprep(np.asarray(inputs["bq"], np.float32) * 0.125),
        "b1": vprep(inputs["b1"]),
        "g1": vprep(inputs["gamma1"]), "be1": vprep(inputs["beta1"]),
        "g2": vprep(inputs["gamma2"]), "be2": vprep(inputs["beta2"]),
    }

    in_maps = []
    for core in range(N_CORES):
        sl = seq[core * BL:(core + 1) * BL].reshape(T).astype(np.int64)
        onehot = np.zeros((KV * P, T), np.float32)
        tok = np.arange(T)
        onehot[sl, tok] = 1.0                       # vocab row
        onehot[V + (tok % S), tok] = 1.0            # position row
        onehot = np.ascontiguousarray(
            onehot.reshape(KV, P, T).transpose(1, 0, 2)).astype(_BF16)
        m = dict(shared)
        m["onehot"] = onehot
        in_maps.append(m)
    return in_maps


def _run(inputs, trace=False):
    from concourse import bass_utils
    if "nc" not in _cache:
        _cache["nc"] = _build_bass()
    nc = _cache["nc"]
    in_maps = _host_prep(inputs)
    res = bass_utils.run_bass_kernel_spmd(
        nc, in_maps, core_ids=list(range(N_CORES)), trace=trace)
    outs = [r["out"].reshape(BL, S, D) for r in res.results]
    full = np.concatenate(outs, axis=0).astype(np.float32)
    return full, res


def kernel(**inputs) -> np.ndarray:
    full, _ = _run(inputs, trace=False)
    return full



# revision 21
# speedup vs baseline: 1.8561x; 1.2329x over previous
"""Trainium2 Bass kernel for a 4-layer dense transformer encoder with BatchNorm.

Model (from reference):
  B=128, S=256, D=512, L=4, V=96, H=8, FF=512, DH=64, eps=1e-3
  x = embed[sequence] + pos
  per layer: MHA -> BN(h+attn) -> FFN(relu) -> BN(h+ffn)   (BN in training mode,
  stats over (batch, seq) per feature)

Sharding: data-parallel over batch across 8 cores (16 batches / core).
BN stats are all-reduced (sum, sumsq per feature = 4KB) across cores.

Device layout: activations are kept feature-major ("transposed"):
  hT[feat, token] with feat on partitions (4 tiles of 128) and 4096 tokens free.
All six projections per layer are then natural PE matmuls
  (lhsT = W[feat_in, feat_out], rhs = hT) and BN per-feature scalars are
per-partition tensor_scalar ops.

Attention per (batch, head): scores psum[q=128, k=256] = qT^T kT (K=DH=64,
row-group packed for even/odd heads); exp on ScalarE with accumulated row sums
(no max-subtraction needed: |scores| <~ 1 by construction). Softmax
normalization is a per-partition (per-q) in-place multiply of P by 1/rowsum
(one broadcast tensor_tensor per head pair); the transpose of P needed for
P@V is then a plain identity-moving matmul; P^T feeds
attnT[dh, q] = V-lhsT matmuls (col-group packed head pairs).

Both BN affines are folded into the adjacent matmuls rather than applied as
elementwise passes: a2/c2 go into the QKV weights (scaled in-place on device;
Q-bias corrected by a small W^T c matvec; the K/V corrections are provably
softmax/BN-invariant and dropped) and the residual+BN becomes an extra
diag(a) matmul accumulated into the O-proj / W2 psum with the +c added by the
psum-drain copy. rstd is computed on VectorE only (quake rsqrt + 2 Newton
steps) so ScalarE keeps a single activation table (exp) all run long.

Matmul inputs are bf16 (fp32 PSUM accumulate); the residual stream hT stays
fp32. The embedding gather runs on-device as a one-hot matmul: the host builds
a sparse one-hot (vocab + seq-position rows) and the kernel contracts it with
[embed; pos_encodings]. The final BN2-apply + [feat,tok]->[tok,feat]
transpose is fused into per-tile fp32 matmuls (diag(a2) + rank-1 ones x c2row)
feeding contiguous output DMAs.

replicate=N builds a NEFF that runs the whole computation N times
back-to-back (per-rep independent; used only for timing: the delta between
replicate=N and replicate=1 walls measures (N-1) x true exec with the
host/tunnel overhead cancelling exactly).
"""

import numpy as np
import ml_dtypes

# ---------------------------------------------------------------- constants
B, S, D, L, V, H, FF = 128, 256, 512, 4, 96, 8, 512
DH = D // H
EPS = 1e-3
N_CORES = 8
BL = B // N_CORES          # local batches per core
T = BL * S                 # local tokens per core = 4096
P = 128                    # partitions
NF = D // P                # feature tiles = 4
CH = 512                   # token chunk
NCH = T // CH              # chunks = 8
KV = 3                     # one-hot contraction tiles (384 rows / 128)
NT = B * S                 # global token count for BN stats

_BF16 = ml_dtypes.bfloat16

_cache = {}

# tile-pool buffer counts (tunable)
POOL_CFG = dict(cpool1=2, cpool2=2, ppool=8, dpool=16, tpool=8, spool=8,
                opool=3, fpool=2, psA=2, psS=3, psT=2, psV=1)

# engine assignment for contested drains/copies: "v"=DVE, "g"=Pool, "s"=ScalarE
# norm: "diag_g"/"diag_v" = build diag(1/rowsum) on Pool/DVE (moving operand of
# the P-transpose matmul); "norm_v"/"norm_g" = normalize P in place instead.
ENG_CFG = dict(ptb=("v", "v"), avl="v", hbf=("v", "v"), norm="diag_v",
               recip="pair")


def _build_bass(sim=False, boring_final=False, sqrt_rstd=False,
                no_collective=False, replicate=1, eng_cfg=None):
    """Build the Bass program. sim=True builds a single-core variant with the
    AllReduce replaced by a local DRAM copy (for TimelineSim cost analysis)."""
    import concourse.bacc as bacc
    import concourse.tile as tile
    from concourse import mybir
    from concourse.masks import make_identity

    ecfg = dict(ENG_CFG)
    if eng_cfg:
        ecfg.update(eng_cfg)

    f32 = mybir.dt.float32
    bf16 = mybir.dt.bfloat16
    Alu = mybir.AluOpType
    Act = mybir.ActivationFunctionType

    nc = bacc.Bacc("TRN2", target_bir_lowering=False, debug=False,
                   num_devices=1 if sim else N_CORES)

    # ------------------------------------------------------------ dram I/O
    onehot_d = nc.dram_tensor("onehot", [P, KV, T], bf16,
                              kind="ExternalInput").ap()
    embt_d = nc.dram_tensor("embt", [P, KV, D], bf16,
                            kind="ExternalInput").ap()
    w_d = {}
    for name in ("wq", "wk", "wv", "wo", "w1", "w2"):
        w_d[name] = nc.dram_tensor(name, [L, P, NF, D], bf16,
                                   kind="ExternalInput").ap()
    vec_d = {}
    for name in ("bq", "b1", "g1", "be1", "g2", "be2"):
        vec_d[name] = nc.dram_tensor(name, [L, P, NF], f32,
                                     kind="ExternalInput").ap()
    out_d = nc.dram_tensor("out", [T, D], f32, kind="ExternalOutput").ap()

    def eng_of(code):
        return {"v": nc.vector, "g": nc.gpsimd, "s": nc.scalar}[code]

    def copy_on(code, out, in_):
        if code == "s":
            nc.scalar.copy(out=out, in_=in_)
        else:
            eng_of(code).tensor_copy(out=out, in_=in_)

    def affine_cast_on(code, out, in_, a_ap, c_ap):
        """out = a*in_ + c with per-partition a, c (BN apply fused into the
        bf16 cast)."""
        if code == "s":
            nc.scalar.activation(out=out, in_=in_,
                                 func=mybir.ActivationFunctionType.Identity,
                                 scale=a_ap, bias=c_ap)
        else:
            eng_of(code).tensor_scalar(out=out, in0=in_,
                                       scalar1=a_ap, scalar2=c_ap,
                                       op0=mybir.AluOpType.mult,
                                       op1=mybir.AluOpType.add)

    with tile.TileContext(nc) as tc:
        from contextlib import ExitStack
        ctx = ExitStack()
        with ctx:
            const = ctx.enter_context(tc.tile_pool(name="const", bufs=1))
            hpool = ctx.enter_context(tc.tile_pool(name="h", bufs=1))
            wpool = ctx.enter_context(tc.tile_pool(name="w", bufs=2))
            wpool1 = ctx.enter_context(tc.tile_pool(name="w1p", bufs=1))
            bpool = ctx.enter_context(tc.tile_pool(name="bias", bufs=2))
            stat = ctx.enter_context(tc.tile_pool(name="stat", bufs=2))
            dramp = ctx.enter_context(tc.tile_pool(name="dramp", bufs=2,
                                                   space="DRAM"))

            hT = hpool.tile([P, NF, T], f32)

            ident_bf = const.tile([P, P], bf16)
            make_identity(nc, ident_bf)
            ident_f32 = const.tile([P, P], f32)
            make_identity(nc, ident_f32)
            eps_sb = const.tile([P, 1], f32)
            nc.vector.memset(eps_sb, EPS)
            ones_f32 = const.tile([1, P], f32)
            nc.vector.memset(ones_f32, 1.0)
            magic_sb = const.tile([P, NF], mybir.dt.uint32)
            nc.vector.memset(magic_sb, 0x5F3759DF)
            one_u32 = const.tile([P, NF], mybir.dt.uint32)
            nc.vector.memset(one_u32, 1)

            # ------------------------------------------------ layer pools
            cpool1 = ctx.enter_context(tc.tile_pool(name="chunk1", bufs=POOL_CFG["cpool1"]))
            cpool2 = ctx.enter_context(tc.tile_pool(name="chunk2", bufs=POOL_CFG["cpool2"]))
            ppool = ctx.enter_context(tc.tile_pool(name="attn", bufs=POOL_CFG["ppool"]))
            dpool = ctx.enter_context(tc.tile_pool(name="diag", bufs=POOL_CFG["dpool"]))
            tpool = ctx.enter_context(tc.tile_pool(name="ptb", bufs=POOL_CFG["tpool"]))
            spool = ctx.enter_context(tc.tile_pool(name="small", bufs=POOL_CFG["spool"]))
            opool = ctx.enter_context(tc.tile_pool(name="outp", bufs=POOL_CFG["opool"]))
            fpool = ctx.enter_context(tc.tile_pool(name="fold", bufs=POOL_CFG["fpool"]))
            psA = ctx.enter_context(tc.tile_pool(name="psA", bufs=POOL_CFG["psA"],
                                                 space="PSUM"))
            psS = ctx.enter_context(tc.tile_pool(name="psS", bufs=POOL_CFG["psS"],
                                                 space="PSUM"))
            psT = ctx.enter_context(tc.tile_pool(name="psT", bufs=POOL_CFG["psT"],
                                                 space="PSUM"))
            psV = ctx.enter_context(tc.tile_pool(name="psV", bufs=POOL_CFG["psV"],
                                                 space="PSUM"))

            def bn_allreduce(stats_tile, g_sb, be_sb, tag, sfx):
                """stats_tile [P, NF, NCH, 6] -> per-feature affine (a, c):
                bn_out = a * z + c, with global (all-core) stats."""
                mv = stat.tile([P, NF, 2], f32, tag=tag + "mv")
                for f in range(NF):
                    nc.vector.bn_aggr(out=mv[:, f, :], in_=stats_tile[:, f, :, :])
                ss = stat.tile([P, NF, 2], f32, tag=tag + "ss")
                tmp = stat.tile([P, NF], f32, tag=tag + "tmp")
                # local sum = mean * T
                nc.vector.tensor_scalar_mul(ss[:, :, 0], mv[:, :, 0], float(T))
                # local sumsq = (var + mean^2) * T
                nc.vector.tensor_tensor(tmp[:], mv[:, :, 0], mv[:, :, 0],
                                        Alu.mult)
                nc.vector.tensor_tensor(tmp[:], tmp[:], mv[:, :, 1], Alu.add)
                # eps folded in here (pre-barrier, off the critical path):
                # after the allreduce, msq - mu^2 = var + eps directly
                nc.vector.tensor_scalar(out=tmp[:], in0=tmp[:],
                                        scalar1=EPS / N_CORES, scalar2=float(T),
                                        op0=Alu.add, op1=Alu.mult)
                nc.vector.tensor_copy(out=ss[:, :, 1], in_=tmp[:])
                din = dramp.tile([P, NF, 2], f32, tag=tag + "din")
                dout = dramp.tile([P, NF, 2], f32, tag=tag + "dout")
                nc.sync.dma_start(din[:], ss[:])
                if sim or no_collective:
                    nc.sync.dma_start(dout[:], din[:])
                else:
                    nc.gpsimd.collective_compute(
                        "AllReduce", Alu.add,
                        replica_groups=[list(range(N_CORES))],
                        ins=[din.opt()], outs=[dout.opt()])
                gs = stat.tile([P, NF, 2], f32, tag=tag + "gs")
                nc.sync.dma_start(gs[:], dout[:])
                mu = stat.tile([P, NF], f32, tag=tag + "mu")
                var = stat.tile([P, NF], f32, tag=tag + "var")
                nt = float(T if (sim or no_collective) else NT)
                nc.vector.tensor_scalar_mul(mu[:], gs[:, :, 0], 1.0 / nt)
                nc.vector.tensor_scalar_mul(var[:], gs[:, :, 1], 1.0 / nt)
                nc.vector.tensor_tensor(tmp[:], mu[:], mu[:], Alu.mult)
                nc.vector.tensor_tensor(var[:], var[:], tmp[:], Alu.subtract)
                if sqrt_rstd:
                    nc.scalar.activation(out=var[:], in_=var[:],
                                         func=Act.Sqrt, bias=eps_sb[:])
                    nc.vector.reciprocal(var[:], var[:])
                else:
                    # rstd = 1/sqrt(var+eps) via quake rsqrt + 2 Newton steps,
                    # entirely on DVE — keeps ScalarE's table set pinned to
                    # exp (no ACT_TABLE_LOAD churn on the BN critical path).
                    # (eps already folded into the all-reduced sumsq)
                    y = stat.tile([P, NF], f32, tag=tag + "y")
                    yi = y.bitcast(mybir.dt.uint32)
                    nc.vector.tensor_tensor(
                        yi[:], var.bitcast(mybir.dt.uint32)[:], one_u32[:],
                        Alu.logical_shift_right)
                    nc.vector.tensor_tensor(yi[:], magic_sb[:], yi[:],
                                            Alu.subtract)
                    t2 = stat.tile([P, NF], f32, tag=tag + "t2")
                    for _ in range(2):
                        nc.vector.tensor_tensor(t2[:], y[:], y[:], Alu.mult)
                        nc.vector.tensor_tensor(t2[:], t2[:], var[:], Alu.mult)
                        nc.vector.tensor_scalar(out=t2[:], in0=t2[:],
                                                scalar1=-0.5, scalar2=1.5,
                                                op0=Alu.mult, op1=Alu.add)
                        nc.vector.tensor_tensor(y[:], y[:], t2[:], Alu.mult)
                    nc.vector.tensor_copy(out=var[:], in_=y[:])
                a = stat.tile([P, NF], f32, tag=tag + "a")
                c = stat.tile([P, NF], f32, tag=tag + "c")
                nc.vector.tensor_tensor(a[:], var[:], g_sb[:], Alu.mult)
                nc.vector.tensor_tensor(c[:], mu[:], a[:], Alu.mult)
                nc.vector.tensor_tensor(c[:], be_sb[:], c[:], Alu.subtract)
                return a, c

            def build_diag(a, tag, dt=None, ident=None):
                """diagA[:, f, :] = diag(a[:, f]), for residual+BN folding
                via PE matmul."""
                dA = fpool.tile([P, NF, P], dt or bf16, tag="diagA",
                                name=f"diagA_{tag}")
                idt = ident or ident_bf
                nc.vector.tensor_tensor(
                    dA[:], idt[:, None, :].to_broadcast((P, NF, P)),
                    a[:, :, None].to_broadcast((P, NF, P)), Alu.mult)
                return dA

            for rep in range(replicate):
                sfx = f"_r{rep}" if replicate > 1 else ""

                # ------------------------------------------------ embedding
                # one-hot streamed in chunk tiles (double-buffered) so the
                # embedding section coexists with the resident layer pools
                with tc.tile_pool(name=f"embp{sfx}", bufs=1) as epool, \
                     tc.tile_pool(name=f"ohp{sfx}", bufs=3) as ohpool:
                    emb = epool.tile([P, KV, D], bf16)
                    nc.sync.dma_start(emb[:], embt_d[:])
                    for t8 in range(NCH):
                        oh = ohpool.tile([P, KV, CH], bf16, tag="oh")
                        nc.sync.dma_start(oh[:],
                                          onehot_d[:, :, t8 * CH:(t8 + 1) * CH])
                        for f in range(NF):
                            ps = psA.tile([P, CH], f32, tag="psA")
                            for kc in range(KV):
                                nc.tensor.matmul(
                                    ps, lhsT=emb[:, kc, f * P:(f + 1) * P],
                                    rhs=oh[:, kc, :],
                                    start=(kc == 0), stop=(kc == KV - 1))
                            dst = hT[:, f, t8 * CH:(t8 + 1) * CH]
                            if (t8 * NF + f) % 2 == 0:
                                nc.vector.tensor_copy(out=dst, in_=ps)
                            else:
                                nc.scalar.copy(out=dst, in_=ps)

                a_pend, c_pend = None, None

                for l in range(L):
                    w = {}
                    for name in ("wq", "wk", "wv", "wo", "w1", "w2"):
                        pool = wpool if name in ("wq", "wk", "wv") else wpool1
                        w[name] = pool.tile([P, NF, D], bf16, tag=name,
                                            name=f"{name}_l{l}{sfx}")
                        nc.sync.dma_start(w[name][:], w_d[name][l])
                    vec = {}
                    for name in ("bq", "b1", "g1", "be1", "g2", "be2"):
                        vec[name] = bpool.tile([P, NF], f32, tag=name,
                                               name=f"{name}_l{l}{sfx}")
                        nc.sync.dma_start(vec[name][:], vec_d[name][l])

                    bqtot = vec["bq"]

                    # =================== phase A: attention ===================
                    # The previous BN2 (a_pend, c_pend) is applied inside the
                    # bf16 cast of the residual stream: hbf = a2*z + c2 =
                    # BN2(z) — same per-element cost as a plain cast, and the
                    # residual accumulate becomes a plain identity matmul.
                    stats1 = stat.tile([P, NF, NCH, 6], f32, tag="st1")
                    for c in range(NCH):
                        tsl = slice(c * CH, (c + 1) * CH)
                        hbf = cpool2.tile([P, NF, CH], bf16, tag="hbf")
                        for f in range(NF):
                            if a_pend is None:
                                copy_on(ecfg["hbf"][0], hbf[:, f, :],
                                        hT[:, f, tsl])
                            else:
                                affine_cast_on(
                                    ecfg["hbf"][0], hbf[:, f, :],
                                    hT[:, f, tsl], a_pend[:, f:f + 1],
                                    c_pend[:, f:f + 1])

                        # --- Q, K (transposed), V (token-major) projections
                        qT = cpool1.tile([P, NF, CH], bf16, tag="qT")
                        kT = cpool1.tile([P, NF, CH], bf16, tag="kT")
                        vU = cpool1.tile([P, NF, CH], bf16, tag="vU")
                        for f in range(NF):
                            ps = psA.tile([P, CH], f32, tag="psA")
                            for kc in range(NF):
                                nc.tensor.matmul(
                                    ps, lhsT=w["wq"][:, kc, f * P:(f + 1) * P],
                                    rhs=hbf[:, kc, :],
                                    start=(kc == 0), stop=(kc == NF - 1))
                            nc.vector.tensor_scalar(
                                out=qT[:, f, :], in0=ps, scalar1=0.125,
                                scalar2=bqtot[:, f:f + 1],
                                op0=Alu.mult, op1=Alu.add)
                            ps = psA.tile([P, CH], f32, tag="psA")
                            for kc in range(NF):
                                nc.tensor.matmul(
                                    ps, lhsT=w["wk"][:, kc, f * P:(f + 1) * P],
                                    rhs=hbf[:, kc, :],
                                    start=(kc == 0), stop=(kc == NF - 1))
                            nc.vector.tensor_copy(out=kT[:, f, :], in_=ps)
                        for ts in range(4):
                            ps = psA.tile([P, CH], f32, tag="psA")
                            for kc in range(NF):
                                nc.tensor.matmul(
                                    ps, lhsT=hbf[:, kc, ts * P:(ts + 1) * P],
                                    rhs=w["wv"][:, kc, :],
                                    start=(kc == 0), stop=(kc == NF - 1))
                            nc.vector.tensor_copy(out=vU[:, ts, :], in_=ps)

                        # --- attention for the two batches of this chunk
                        attnT = cpool1.tile([P, NF, CH], bf16, tag="attnT")
                        for bb in range(2):
                            boff = bb * S
                            Ps = {}
                            rrs = [None, None]
                            for qt in range(2):
                                rs = spool.tile([P, H], f32, tag="rs",
                                                name=f"rs_{c}_{bb}_{qt}{sfx}")
                                rr = spool.tile([P, H], f32, tag="rr",
                                                name=f"rr_{c}_{bb}_{qt}{sfx}")
                                for h in range(H):
                                    if h % 2 == 0:
                                        # per-head-pair P tile: finer lifetime
                                        # -> deeper cross-chunk pipelining
                                        Pp = ppool.tile(
                                            [P, 2, S], bf16, tag="P",
                                            name=f"P_{c}_{bb}_{qt}_{h // 2}{sfx}")
                                        Ps[qt, h // 2] = Pp
                                    po = (h % 2) * DH
                                    fi = h // 2
                                    sps = psS.tile([P, S], f32, tag="sps")
                                    nc.tensor.matmul(
                                        sps,
                                        lhsT=qT[po:po + DH, fi,
                                                boff + qt * P:boff + (qt + 1) * P],
                                        rhs=kT[po:po + DH, fi, boff:boff + S],
                                        start=True, stop=True)
                                    # P = exp(scores) (1/8 folded into qT),
                                    # rowsum accumulated per head
                                    nc.scalar.activation(
                                        out=Ps[qt, h // 2][:, h % 2, :],
                                        in_=sps, func=Act.Exp,
                                        accum_out=rs[:, h:h + 1])
                                    if h % 2 == 1 and ecfg["recip"] == "pair":
                                        # per head-pair: shortens the exp->diag
                                        # join from 8 exps to 2
                                        nc.vector.reciprocal(
                                            rr[:, h - 1:h + 1],
                                            rs[:, h - 1:h + 1])
                                if ecfg["recip"] == "qt":
                                    nc.vector.reciprocal(rr[:], rs[:])
                                rrs[qt] = rr
                            # softmax-normalize + transpose: either normalize P
                            # rows in place (per-q 1/rowsum) and transpose with
                            # an identity moving operand, or keep P raw and use
                            # diag(1/rowsum) as the moving operand.
                            nmode = ecfg["norm"]
                            for h in range(H):
                                fi = h // 2
                                if nmode.startswith("norm") and h % 2 == 0:
                                    neng = (nc.vector if nmode == "norm_v"
                                            else nc.gpsimd)
                                    for qt in range(2):
                                        neng.tensor_tensor(
                                            Ps[qt, fi][:], Ps[qt, fi][:],
                                            rrs[qt][:, 2 * fi:2 * fi + 2, None]
                                            .to_broadcast((P, 2, S)),
                                            Alu.mult)
                                if nmode.startswith("diag"):
                                    deng = (nc.vector if nmode == "diag_v"
                                            else nc.gpsimd)
                                    diag = [None, None]
                                    for qt in range(2):
                                        dg = dpool.tile(
                                            [P, P], bf16, tag="dg",
                                            name=f"dg_{c}_{bb}_{h}_{qt}{sfx}")
                                        deng.tensor_scalar_mul(
                                            dg[:], ident_bf[:],
                                            rrs[qt][:, h:h + 1])
                                        diag[qt] = dg
                                ptb = tpool.tile([P, 2, S], bf16, tag="ptb")
                                pt_ps = psT.tile([P, 2 * S], f32, tag="ptps")
                                for kc in range(2):
                                    for qt in range(2):
                                        nc.tensor.matmul(
                                            pt_ps[:, kc * S + qt * P:
                                                  kc * S + (qt + 1) * P],
                                            lhsT=Ps[qt, fi][:, h % 2,
                                                            kc * P:(kc + 1) * P],
                                            rhs=(ident_bf[:]
                                                 if nmode.startswith("norm")
                                                 else diag[qt][:]),
                                            start=True, stop=True)
                                copy_on(ecfg["ptb"][h % 2], ptb[:], pt_ps)
                                if h % 4 == 0:
                                    avl = psV.tile([P, 2 * S], f32, tag="avps")
                                hh = h % 2
                                jj = (h % 4) // 2
                                for kc in range(2):
                                    nc.tensor.matmul(
                                        avl[hh * DH:(hh + 1) * DH,
                                            jj * S:(jj + 1) * S],
                                        lhsT=vU[:, 2 * bb + kc,
                                                h * DH:(h + 1) * DH],
                                        rhs=ptb[:, kc, :],
                                        start=(kc == 0), stop=(kc == 1))
                                if h % 4 == 3:
                                    j2 = h // 4  # f-tile pair index (0 or 1)
                                    dst = attnT[:, 2 * j2:2 * j2 + 2,
                                                boff:boff + S]
                                    copy_on(ecfg["avl"], dst, avl)
                        # --- O-projection + residual: psum = Wo^T attnT + z2
                        # (hbf already carries BN2(z_prev)).
                        for f in range(NF):
                            ps = psA.tile([P, CH], f32, tag="psA")
                            for kc in range(NF):
                                nc.tensor.matmul(
                                    ps, lhsT=w["wo"][:, kc, f * P:(f + 1) * P],
                                    rhs=attnT[:, kc, :],
                                    start=(kc == 0), stop=False)
                            nc.tensor.matmul(
                                ps, lhsT=ident_bf[:], rhs=hbf[:, f, :],
                                start=False, stop=True)
                            nc.scalar.copy(out=hT[:, f, tsl], in_=ps)
                            nc.vector.bn_stats(out=stats1[:, f, c, :],
                                               in_=hT[:, f, tsl])

                    a1, c1 = bn_allreduce(stats1, vec["g1"], vec["be1"],
                                          "bn1", sfx)

                    # ======================= phase B: FFN =====================
                    # BN1 is applied inside the bf16 cast: h1bf = BN1(z1);
                    # the residual is then a plain identity accumulate.
                    stats2 = stat.tile([P, NF, NCH, 6], f32, tag="st2")
                    for c in range(NCH):
                        tsl = slice(c * CH, (c + 1) * CH)
                        h1bf = cpool2.tile([P, NF, CH], bf16, tag="hbf",
                                           name=f"h1bf_{l}_{c}{sfx}")
                        for f in range(NF):
                            affine_cast_on(
                                ecfg["hbf"][1], h1bf[:, f, :], hT[:, f, tsl],
                                a1[:, f:f + 1], c1[:, f:f + 1])
                        ffn = cpool2.tile([P, NF, CH], bf16, tag="ffn")
                        for f in range(NF):
                            ps = psA.tile([P, CH], f32, tag="psA")
                            for kc in range(NF):
                                nc.tensor.matmul(
                                    ps, lhsT=w["w1"][:, kc, f * P:(f + 1) * P],
                                    rhs=h1bf[:, kc, :],
                                    start=(kc == 0), stop=(kc == NF - 1))
                            nc.scalar.activation(out=ffn[:, f, :], in_=ps,
                                                 func=Act.Relu,
                                                 bias=vec["b1"][:, f:f + 1])
                        # W2 + residual: psum = W2^T ffn + h1bf (= BN1(z1))
                        for f in range(NF):
                            ps = psA.tile([P, CH], f32, tag="psA")
                            for kc in range(NF):
                                nc.tensor.matmul(
                                    ps, lhsT=w["w2"][:, kc, f * P:(f + 1) * P],
                                    rhs=ffn[:, kc, :],
                                    start=(kc == 0), stop=False)
                            nc.tensor.matmul(
                                ps, lhsT=ident_bf[:], rhs=h1bf[:, f, :],
                                start=False, stop=True)
                            nc.scalar.copy(out=hT[:, f, tsl], in_=ps)
                            nc.vector.bn_stats(out=stats2[:, f, c, :],
                                               in_=hT[:, f, tsl])

                    a_pend, c_pend = bn_allreduce(stats2, vec["g2"],
                                                  vec["be2"], "bn2", sfx)

                # ======== final: fused BN2-apply + transpose + store ========
                # out[t, d] = a2[d]*z[d, t] + c2[d], via matmul with diag(a2)
                # plus a rank-1 (ones x c2row) accumulate.
                if boring_final:
                    for c in range(NCH):
                        tsl = slice(c * CH, (c + 1) * CH)
                        for f in range(NF):
                            nc.vector.tensor_scalar(
                                out=hT[:, f, tsl], in0=hT[:, f, tsl],
                                scalar1=a_pend[:, f:f + 1],
                                scalar2=c_pend[:, f:f + 1],
                                op0=Alu.mult, op1=Alu.add)
                    diagAF = crow = None
                else:
                    diagAF = build_diag(a_pend, f"final{sfx}", dt=f32,
                                        ident=ident_f32)
                    crow_ps = psS.tile([1, 2 * S], f32, tag="sps",
                                       name=f"crow_ps{sfx}")
                    for f in range(NF):
                        nc.tensor.matmul(
                            crow_ps[0:1, f * P:(f + 1) * P],
                            lhsT=c_pend[:, f:f + 1], rhs=ident_f32[:],
                            start=True, stop=True)
                    crow = const.tile([1, NF * P], f32)
                    nc.vector.tensor_copy(out=crow[:],
                                          in_=crow_ps[0:1, :NF * P])
                for tt in range(T // P):
                    ops = psA.tile([P, CH], f32, tag="psA")
                    for f in range(NF):
                        if boring_final:
                            nc.tensor.matmul(
                                ops[:, f * P:(f + 1) * P],
                                lhsT=hT[:, f, tt * P:(tt + 1) * P],
                                rhs=ident_f32[:],
                                start=True, stop=True)
                            continue
                        nc.tensor.matmul(
                            ops[:, f * P:(f + 1) * P],
                            lhsT=hT[:, f, tt * P:(tt + 1) * P],
                            rhs=diagAF[:, f, :],
                            start=True, stop=False)
                        nc.tensor.matmul(
                            ops[:, f * P:(f + 1) * P],
                            lhsT=ones_f32[:], rhs=crow[0:1, f * P:(f + 1) * P],
                            start=False, stop=True)
                    ot = opool.tile([P, CH], f32, tag="ot")
                    if tt % 2 == 0:
                        nc.vector.tensor_copy(out=ot[:], in_=ops)
                    else:
                        nc.scalar.copy(out=ot[:], in_=ops)
                    nc.sync.dma_start(out_d[tt * P:(tt + 1) * P, :], ot[:])

    nc.compile()
    return nc


def _host_prep(inputs):
    """Build per-core in_maps from the full inputs."""
    seq = np.asarray(inputs["sequence"])
    pos = np.asarray(inputs["pos_encodings"], dtype=np.float32)
    emb = np.asarray(inputs["embed"], dtype=np.float32)

    # extended embedding table: rows 0..95 vocab, 96..351 positions, pad to 384
    embt = np.zeros((KV * P, D), np.float32)
    embt[:V] = emb
    embt[V:V + S] = pos
    embt = np.ascontiguousarray(
        embt.reshape(KV, P, D).transpose(1, 0, 2))          # [P, KV, D]

    def wprep(wa):  # [L, D, X] -> [L, P, NF, X] bf16
        wa = np.asarray(wa, dtype=np.float32)
        return np.ascontiguousarray(
            wa.reshape(L, NF, P, wa.shape[-1]).transpose(0, 2, 1, 3)
        ).astype(_BF16)

    def vprep(va):  # [L, D] -> [L, P, NF] f32
        va = np.asarray(va, dtype=np.float32)
        return np.ascontiguousarray(va.reshape(L, NF, P).transpose(0, 2, 1))

    shared = {
        "embt": embt.astype(_BF16),
        "wq": wprep(inputs["Wq"]), "wk": wprep(inputs["Wk"]),
        "wv": wprep(inputs["Wv"]), "wo": wprep(inputs["Wo"]),
        "w1": wprep(inputs["W1"]), "w2": wprep(inputs["W2"]),
        # scores are scaled by 1/sqrt(DH)=1/8 during the Q copy via
        # activation(scale=0.125), which computes in*scale + bias — so the
        # q bias must be pre-scaled here.
        "bq": vprep(np.asarray(inputs["bq"], np.float32) * 0.125),
        "b1": vprep(inputs["b1"]),
        "g1": vprep(inputs["gamma1"]), "be1": vprep(inputs["beta1"]),
        "g2": vprep(inputs["gamma2"]), "be2": vprep(inputs["beta2"]),
    }

    in_maps = []
    for core in range(N_CORES):
        sl = seq[core * BL:(core + 1) * BL].reshape(T).astype(np.int64)
        onehot = np.zeros((KV * P, T), np.float32)
        tok = np.arange(T)
        onehot[sl, tok] = 1.0                       # vocab row
        onehot[V + (tok % S), tok] = 1.0            # position row
        onehot = np.ascontiguousarray(
            onehot.reshape(KV, P, T).transpose(1, 0, 2)).astype(_BF16)
        m = dict(shared)
        m["onehot"] = onehot
        in_maps.append(m)
    return in_maps


def _run(inputs, trace=False):
    from concourse import bass_utils
    if "nc" not in _cache:
        _cache["nc"] = _build_bass()
    nc = _cache["nc"]
    in_maps = _host_prep(inputs)
    res = bass_utils.run_bass_kernel_spmd(
        nc, in_maps, core_ids=list(range(N_CORES)), trace=trace)
    outs = [r["out"].reshape(BL, S, D) for r in res.results]
    full = np.concatenate(outs, axis=0).astype(np.float32)
    return full, res


def kernel(**inputs) -> np.ndarray:
    full, _ = _run(inputs, trace=False)
    return full


# revision 29
# speedup vs baseline: 5.3208x; 2.8667x over previous
"""Trainium2 Bass kernel for a 4-layer dense transformer encoder with BatchNorm.

Model (from reference):
  B=128, S=256, D=512, L=4, V=96, H=8, FF=512, DH=64, eps=1e-3
  x = embed[sequence] + pos
  per layer: MHA -> BN(h+attn) -> FFN(relu) -> BN(h+ffn)   (BN in training mode,
  stats over (batch, seq) per feature)

Sharding: data-parallel over batch across 8 cores (16 batches / core).
BN stats are all-reduced (sum, sumsq per feature = 4KB) across cores.

Device layout: activations are kept feature-major ("transposed"):
  hT[feat, token] with feat on partitions (4 tiles of 128) and 4096 tokens free.
All six projections per layer are then natural PE matmuls
  (lhsT = W[feat_in, feat_out], rhs = hT) and BN per-feature scalars are
per-partition tensor_scalar ops.

Attention per (batch, head): scores psum[q=128, k=256] = qT^T kT (K=DH=64,
row-group packed for even/odd heads); exp on ScalarE with accumulated row sums
(no max-subtraction needed: |scores| <~ 1 by construction). Softmax
normalization is a per-partition (per-q) in-place multiply of P by 1/rowsum
(one broadcast tensor_tensor per head pair); the transpose of P needed for
P@V is then a plain identity-moving matmul; P^T feeds
attnT[dh, q] = V-lhsT matmuls (col-group packed head pairs).

Both BN affines are folded into the adjacent matmuls rather than applied as
elementwise passes: a2/c2 go into the QKV weights (scaled in-place on device;
Q-bias corrected by a small W^T c matvec; the K/V corrections are provably
softmax/BN-invariant and dropped) and the residual+BN becomes an extra
diag(a) matmul accumulated into the O-proj / W2 psum with the +c added by the
psum-drain copy. rstd is computed on VectorE only (quake rsqrt + 2 Newton
steps) so ScalarE keeps a single activation table (exp) all run long.

Matmul inputs are bf16 (fp32 PSUM accumulate); the residual stream hT stays
fp32. The embedding gather runs on-device as a one-hot matmul: the host builds
a sparse one-hot (vocab + seq-position rows) and the kernel contracts it with
[embed; pos_encodings]. The final BN2-apply + [feat,tok]->[tok,feat]
transpose is fused into per-tile fp32 matmuls (diag(a2) + rank-1 ones x c2row)
feeding contiguous output DMAs.

replicate=N builds a NEFF that runs the whole computation N times
back-to-back (per-rep independent; used only for timing: the delta between
replicate=N and replicate=1 walls measures (N-1) x true exec with the
host/tunnel overhead cancelling exactly).
"""

import numpy as np
import ml_dtypes

# ---------------------------------------------------------------- constants
B, S, D, L, V, H, FF = 128, 256, 512, 4, 96, 8, 512
DH = D // H
EPS = 1e-3
N_CORES = 8
BL = B // N_CORES          # local batches per core
T = BL * S                 # local tokens per core = 4096
P = 128                    # partitions
NF = D // P                # feature tiles = 4
CH = 512                   # token chunk
NCH = T // CH              # chunks = 8
KV = 3                     # one-hot contraction tiles (384 rows / 128)
NT = B * S                 # global token count for BN stats

_BF16 = ml_dtypes.bfloat16

_cache = {}

# tile-pool buffer counts (tunable)
POOL_CFG = dict(cpool1=2, cpool2=2, ppool=8, dpool=8, tpool=7, spool=8,
                opool=2, fpool=2, wpool=1, ohp=2, psA=2, psS=3, psT=2, psV=1)

# engine assignment for contested drains/copies: "v"=DVE, "g"=Pool, "s"=ScalarE
# norm: "diag_g"/"diag_v" = build diag(1/rowsum) on Pool/DVE (moving operand of
# the P-transpose matmul); "norm_v"/"norm_g" = normalize P in place instead.
ENG_CFG = dict(ptb=("v", "v"), avl="v", hbf=("v", "v"), norm="diag_v",
               recip="pair")


def _build_bass(sim=False, sqrt_rstd=False,
                no_collective=False, replicate=1, eng_cfg=None):
    """Build the Bass program. sim=True builds a single-core variant with the
    AllReduce replaced by a local DRAM copy (for TimelineSim cost analysis)."""
    import concourse.bacc as bacc
    import concourse.tile as tile
    from concourse import mybir
    from concourse.masks import make_identity

    ecfg = dict(ENG_CFG)
    if eng_cfg:
        ecfg.update(eng_cfg)

    f32 = mybir.dt.float32
    bf16 = mybir.dt.bfloat16
    Alu = mybir.AluOpType
    Act = mybir.ActivationFunctionType

    nc = bacc.Bacc("TRN2", target_bir_lowering=False, debug=False,
                   num_devices=1 if sim else N_CORES)

    # ------------------------------------------------------------ dram I/O
    onehot_d = nc.dram_tensor("onehot", [P, KV, T], bf16,
                              kind="ExternalInput").ap()
    embt_d = nc.dram_tensor("embt", [P, KV, D], bf16,
                            kind="ExternalInput").ap()
    w_d = {}
    for name in ("wq", "wk", "wv", "wo", "w1", "w2"):
        w_d[name] = nc.dram_tensor(name, [L, P, NF, D], bf16,
                                   kind="ExternalInput").ap()
    vec_d = {}
    for name in ("bq", "b1", "g1", "be1", "g2", "be2"):
        vec_d[name] = nc.dram_tensor(name, [L, P, NF], f32,
                                     kind="ExternalInput").ap()
    out_d = nc.dram_tensor("out", [T, D], f32, kind="ExternalOutput").ap()

    def eng_of(code):
        return {"v": nc.vector, "g": nc.gpsimd, "s": nc.scalar}[code]

    def copy_on(code, out, in_):
        if code == "s":
            nc.scalar.copy(out=out, in_=in_)
        else:
            eng_of(code).tensor_copy(out=out, in_=in_)

    def affine_cast_on(code, out, in_, a_ap, c_ap):
        """out = a*in_ + c with per-partition a, c (BN apply fused into the
        bf16 cast)."""
        if code == "s":
            nc.scalar.activation(out=out, in_=in_,
                                 func=mybir.ActivationFunctionType.Identity,
                                 scale=a_ap, bias=c_ap)
        else:
            eng_of(code).tensor_scalar(out=out, in0=in_,
                                       scalar1=a_ap, scalar2=c_ap,
                                       op0=mybir.AluOpType.mult,
                                       op1=mybir.AluOpType.add)

    with tile.TileContext(nc) as tc:
        from contextlib import ExitStack
        ctx = ExitStack()
        with ctx:
            const = ctx.enter_context(tc.tile_pool(name="const", bufs=1))
            hpool = ctx.enter_context(tc.tile_pool(name="h", bufs=1))
            # weights are no longer scaled in place (BN rides the casts), so
            # single-buffering QKV is safe: the next layer's DMA just waits
            # for the last Q/K/V matmul of the current layer.
            wpool = ctx.enter_context(tc.tile_pool(name="w", bufs=POOL_CFG["wpool"]))
            wpool1 = ctx.enter_context(tc.tile_pool(name="w1p", bufs=1))
            bpool = ctx.enter_context(tc.tile_pool(name="bias", bufs=2))
            stat = ctx.enter_context(tc.tile_pool(name="stat", bufs=2))
            dramp = ctx.enter_context(tc.tile_pool(name="dramp", bufs=2,
                                                   space="DRAM"))

            hT = hpool.tile([P, NF, T], f32)
            # last layer's residual stream in bf16: feeds the final
            # BN-apply+transpose matmuls at 1 cycle/row (vs 4 for fp32)
            hTb = hpool.tile([P, NF, T], bf16, tag="hTb")

            ident_bf = const.tile([P, P], bf16)
            make_identity(nc, ident_bf)
            eps_sb = const.tile([P, 1], f32)
            nc.vector.memset(eps_sb, EPS)
            ones_bf = const.tile([1, P], bf16)
            nc.vector.memset(ones_bf, 1.0)
            magic_sb = const.tile([P, NF], mybir.dt.uint32)
            nc.vector.memset(magic_sb, 0x5F3759DF)
            one_u32 = const.tile([P, NF], mybir.dt.uint32)
            nc.vector.memset(one_u32, 1)

            # ------------------------------------------------ layer pools
            cpool1 = ctx.enter_context(tc.tile_pool(name="chunk1", bufs=POOL_CFG["cpool1"]))
            cpool2 = ctx.enter_context(tc.tile_pool(name="chunk2", bufs=POOL_CFG["cpool2"]))
            ppool = ctx.enter_context(tc.tile_pool(name="attn", bufs=POOL_CFG["ppool"]))
            dpool = ctx.enter_context(tc.tile_pool(name="diag", bufs=POOL_CFG["dpool"]))
            tpool = ctx.enter_context(tc.tile_pool(name="ptb", bufs=POOL_CFG["tpool"]))
            spool = ctx.enter_context(tc.tile_pool(name="small", bufs=POOL_CFG["spool"]))
            opool = ctx.enter_context(tc.tile_pool(name="outp", bufs=POOL_CFG["opool"]))
            fpool = ctx.enter_context(tc.tile_pool(name="fold", bufs=POOL_CFG["fpool"]))
            psA = ctx.enter_context(tc.tile_pool(name="psA", bufs=POOL_CFG["psA"],
                                                 space="PSUM"))
            psS = ctx.enter_context(tc.tile_pool(name="psS", bufs=POOL_CFG["psS"],
                                                 space="PSUM"))
            psT = ctx.enter_context(tc.tile_pool(name="psT", bufs=POOL_CFG["psT"],
                                                 space="PSUM"))
            psV = ctx.enter_context(tc.tile_pool(name="psV", bufs=POOL_CFG["psV"],
                                                 space="PSUM"))

            def bn_allreduce(stats_tile, g_sb, be_sb, tag, sfx):
                """stats_tile [P, NF, NCH, 6] -> per-feature affine (a, c):
                bn_out = a * z + c, with global (all-core) stats."""
                mv = stat.tile([P, NF, 2], f32, tag=tag + "mv")
                for f in range(NF):
                    nc.vector.bn_aggr(out=mv[:, f, :], in_=stats_tile[:, f, :, :])
                ss = stat.tile([P, NF, 2], f32, tag=tag + "ss")
                tmp = stat.tile([P, NF], f32, tag=tag + "tmp")
                # local sum = mean * T
                nc.vector.tensor_scalar_mul(ss[:, :, 0], mv[:, :, 0], float(T))
                # local sumsq = (var + mean^2) * T
                nc.vector.tensor_tensor(tmp[:], mv[:, :, 0], mv[:, :, 0],
                                        Alu.mult)
                nc.vector.tensor_tensor(tmp[:], tmp[:], mv[:, :, 1], Alu.add)
                # eps folded in here (pre-barrier, off the critical path):
                # after the allreduce, msq - mu^2 = var + eps directly
                nc.vector.tensor_scalar(out=tmp[:], in0=tmp[:],
                                        scalar1=EPS / N_CORES, scalar2=float(T),
                                        op0=Alu.add, op1=Alu.mult)
                nc.vector.tensor_copy(out=ss[:, :, 1], in_=tmp[:])
                din = dramp.tile([P, NF, 2], f32, tag=tag + "din")
                dout = dramp.tile([P, NF, 2], f32, tag=tag + "dout")
                nc.sync.dma_start(din[:], ss[:])
                if sim or no_collective:
                    nc.sync.dma_start(dout[:], din[:])
                else:
                    nc.gpsimd.collective_compute(
                        "AllReduce", Alu.add,
                        replica_groups=[list(range(N_CORES))],
                        ins=[din.opt()], outs=[dout.opt()])
                gs = stat.tile([P, NF, 2], f32, tag=tag + "gs")
                nc.sync.dma_start(gs[:], dout[:])
                mu = stat.tile([P, NF], f32, tag=tag + "mu")
                var = stat.tile([P, NF], f32, tag=tag + "var")
                nt = float(T if (sim or no_collective) else NT)
                nc.vector.tensor_scalar_mul(mu[:], gs[:, :, 0], 1.0 / nt)
                nc.vector.tensor_scalar_mul(var[:], gs[:, :, 1], 1.0 / nt)
                nc.vector.tensor_tensor(tmp[:], mu[:], mu[:], Alu.mult)
                nc.vector.tensor_tensor(var[:], var[:], tmp[:], Alu.subtract)
                if sqrt_rstd:
                    nc.scalar.activation(out=var[:], in_=var[:],
                                         func=Act.Sqrt, bias=eps_sb[:])
                    nc.vector.reciprocal(var[:], var[:])
                else:
                    # rstd = 1/sqrt(var+eps) via quake rsqrt + 2 Newton steps,
                    # entirely on DVE — keeps ScalarE's table set pinned to
                    # exp (no ACT_TABLE_LOAD churn on the BN critical path).
                    # (eps already folded into the all-reduced sumsq)
                    y = stat.tile([P, NF], f32, tag=tag + "y")
                    yi = y.bitcast(mybir.dt.uint32)
                    nc.vector.tensor_tensor(
                        yi[:], var.bitcast(mybir.dt.uint32)[:], one_u32[:],
                        Alu.logical_shift_right)
                    nc.vector.tensor_tensor(yi[:], magic_sb[:], yi[:],
                                            Alu.subtract)
                    t2 = stat.tile([P, NF], f32, tag=tag + "t2")
                    for _ in range(2):
                        nc.vector.tensor_tensor(t2[:], y[:], y[:], Alu.mult)
                        nc.vector.tensor_tensor(t2[:], t2[:], var[:], Alu.mult)
                        nc.vector.tensor_scalar(out=t2[:], in0=t2[:],
                                                scalar1=-0.5, scalar2=1.5,
                                                op0=Alu.mult, op1=Alu.add)
                        nc.vector.tensor_tensor(y[:], y[:], t2[:], Alu.mult)
                    nc.vector.tensor_copy(out=var[:], in_=y[:])
                a = stat.tile([P, NF], f32, tag=tag + "a")
                c = stat.tile([P, NF], f32, tag=tag + "c")
                nc.vector.tensor_tensor(a[:], var[:], g_sb[:], Alu.mult)
                nc.vector.tensor_tensor(c[:], mu[:], a[:], Alu.mult)
                nc.vector.tensor_tensor(c[:], be_sb[:], c[:], Alu.subtract)
                return a, c

            def build_diag(a, tag, dt=None, ident=None):
                """diagA[:, f, :] = diag(a[:, f]), for residual+BN folding
                via PE matmul."""
                dA = fpool.tile([P, NF, P], dt or bf16, tag="diagA",
                                name=f"diagA_{tag}")
                idt = ident or ident_bf
                nc.vector.tensor_tensor(
                    dA[:], idt[:, None, :].to_broadcast((P, NF, P)),
                    a[:, :, None].to_broadcast((P, NF, P)), Alu.mult)
                return dA

            for rep in range(replicate):
                sfx = f"_r{rep}" if replicate > 1 else ""

                # ------------------------------------------------ embedding
                # one-hot streamed in chunk tiles (double-buffered) so the
                # embedding section coexists with the resident layer pools
                with tc.tile_pool(name=f"embp{sfx}", bufs=1) as epool, \
                     tc.tile_pool(name=f"ohp{sfx}",
                                  bufs=POOL_CFG["ohp"]) as ohpool:
                    emb = epool.tile([P, KV, D], bf16)
                    nc.sync.dma_start(emb[:], embt_d[:])
                    for t8 in range(NCH):
                        oh = ohpool.tile([P, KV, CH], bf16, tag="oh")
                        nc.sync.dma_start(oh[:],
                                          onehot_d[:, :, t8 * CH:(t8 + 1) * CH])
                        for f in range(NF):
                            ps = psA.tile([P, CH], f32, tag="psA")
                            for kc in range(KV):
                                nc.tensor.matmul(
                                    ps, lhsT=emb[:, kc, f * P:(f + 1) * P],
                                    rhs=oh[:, kc, :],
                                    start=(kc == 0), stop=(kc == KV - 1))
                            dst = hT[:, f, t8 * CH:(t8 + 1) * CH]
                            if (t8 * NF + f) % 2 == 0:
                                nc.vector.tensor_copy(out=dst, in_=ps)
                            else:
                                nc.scalar.copy(out=dst, in_=ps)

                a_pend, c_pend = None, None

                for l in range(L):
                    w = {}
                    for name in ("wq", "wk", "wv", "wo", "w1", "w2"):
                        pool = wpool if name in ("wq", "wk", "wv") else wpool1
                        w[name] = pool.tile([P, NF, D], bf16, tag=name,
                                            name=f"{name}_l{l}{sfx}")
                        nc.sync.dma_start(w[name][:], w_d[name][l])
                    vec = {}
                    for name in ("bq", "b1", "g1", "be1", "g2", "be2"):
                        vec[name] = bpool.tile([P, NF], f32, tag=name,
                                               name=f"{name}_l{l}{sfx}")
                        nc.sync.dma_start(vec[name][:], vec_d[name][l])

                    bqtot = vec["bq"]

                    # =================== phase A: attention ===================
                    # The previous BN2 (a_pend, c_pend) is applied inside the
                    # bf16 cast of the residual stream: hbf = a2*z + c2 =
                    # BN2(z) — same per-element cost as a plain cast, and the
                    # residual accumulate becomes a plain identity matmul.
                    stats1 = stat.tile([P, NF, NCH, 6], f32, tag="st1")
                    for c in range(NCH):
                        tsl = slice(c * CH, (c + 1) * CH)
                        hbf = cpool2.tile([P, NF, CH], bf16, tag="hbf")
                        for f in range(NF):
                            if a_pend is None:
                                copy_on(ecfg["hbf"][0], hbf[:, f, :],
                                        hT[:, f, tsl])
                            else:
                                affine_cast_on(
                                    ecfg["hbf"][0], hbf[:, f, :],
                                    hT[:, f, tsl], a_pend[:, f:f + 1],
                                    c_pend[:, f:f + 1])

                        # --- Q, K (transposed), V (token-major) projections
                        qT = cpool1.tile([P, NF, CH], bf16, tag="qT")
                        kT = cpool1.tile([P, NF, CH], bf16, tag="kT")
                        vU = cpool1.tile([P, NF, CH], bf16, tag="vU")
                        for f in range(NF):
                            ps = psA.tile([P, CH], f32, tag="psA")
                            for kc in range(NF):
                                nc.tensor.matmul(
                                    ps, lhsT=w["wq"][:, kc, f * P:(f + 1) * P],
                                    rhs=hbf[:, kc, :],
                                    start=(kc == 0), stop=(kc == NF - 1))
                            nc.vector.tensor_scalar(
                                out=qT[:, f, :], in0=ps, scalar1=0.125,
                                scalar2=bqtot[:, f:f + 1],
                                op0=Alu.mult, op1=Alu.add)
                            ps = psA.tile([P, CH], f32, tag="psA")
                            for kc in range(NF):
                                nc.tensor.matmul(
                                    ps, lhsT=w["wk"][:, kc, f * P:(f + 1) * P],
                                    rhs=hbf[:, kc, :],
                                    start=(kc == 0), stop=(kc == NF - 1))
                            nc.vector.tensor_copy(out=kT[:, f, :], in_=ps)
                        for ts in range(4):
                            ps = psA.tile([P, CH], f32, tag="psA")
                            for kc in range(NF):
                                nc.tensor.matmul(
                                    ps, lhsT=hbf[:, kc, ts * P:(ts + 1) * P],
                                    rhs=w["wv"][:, kc, :],
                                    start=(kc == 0), stop=(kc == NF - 1))
                            nc.vector.tensor_copy(out=vU[:, ts, :], in_=ps)

                        # --- attention for the two batches of this chunk
                        attnT = cpool1.tile([P, NF, CH], bf16, tag="attnT")
                        for bb in range(2):
                            boff = bb * S
                            Ps = {}
                            rrs = [None, None]
                            for qt in range(2):
                                rs = spool.tile([P, H], f32, tag="rs",
                                                name=f"rs_{c}_{bb}_{qt}{sfx}")
                                rr = spool.tile([P, H], f32, tag="rr",
                                                name=f"rr_{c}_{bb}_{qt}{sfx}")
                                for h in range(H):
                                    if h % 2 == 0:
                                        # per-head-pair P tile: finer lifetime
                                        # -> deeper cross-chunk pipelining
                                        Pp = ppool.tile(
                                            [P, 2, S], bf16, tag="P",
                                            name=f"P_{c}_{bb}_{qt}_{h // 2}{sfx}")
                                        Ps[qt, h // 2] = Pp
                                    po = (h % 2) * DH
                                    fi = h // 2
                                    sps = psS.tile([P, S], f32, tag="sps")
                                    nc.tensor.matmul(
                                        sps,
                                        lhsT=qT[po:po + DH, fi,
                                                boff + qt * P:boff + (qt + 1) * P],
                                        rhs=kT[po:po + DH, fi, boff:boff + S],
                                        start=True, stop=True)
                                    # P = exp(scores) (1/8 folded into qT),
                                    # rowsum accumulated per head
                                    nc.scalar.activation(
                                        out=Ps[qt, h // 2][:, h % 2, :],
                                        in_=sps, func=Act.Exp,
                                        accum_out=rs[:, h:h + 1])
                                    if h % 2 == 1 and ecfg["recip"] == "pair":
                                        # per head-pair: shortens the exp->diag
                                        # join from 8 exps to 2
                                        nc.vector.reciprocal(
                                            rr[:, h - 1:h + 1],
                                            rs[:, h - 1:h + 1])
                                if ecfg["recip"] == "qt":
                                    nc.vector.reciprocal(rr[:], rs[:])
                                rrs[qt] = rr
                            # softmax-normalize + transpose: either normalize P
                            # rows in place (per-q 1/rowsum) and transpose with
                            # an identity moving operand, or keep P raw and use
                            # diag(1/rowsum) as the moving operand.
                            nmode = ecfg["norm"]
                            for h in range(H):
                                fi = h // 2
                                if nmode.startswith("norm") and h % 2 == 0:
                                    neng = (nc.vector if nmode == "norm_v"
                                            else nc.gpsimd)
                                    for qt in range(2):
                                        neng.tensor_tensor(
                                            Ps[qt, fi][:], Ps[qt, fi][:],
                                            rrs[qt][:, 2 * fi:2 * fi + 2, None]
                                            .to_broadcast((P, 2, S)),
                                            Alu.mult)
                                if nmode.startswith("diag"):
                                    deng = (nc.vector if nmode == "diag_v"
                                            else nc.gpsimd)
                                    diag = [None, None]
                                    for qt in range(2):
                                        dg = dpool.tile(
                                            [P, P], bf16, tag="dg",
                                            name=f"dg_{c}_{bb}_{h}_{qt}{sfx}")
                                        deng.tensor_scalar_mul(
                                            dg[:], ident_bf[:],
                                            rrs[qt][:, h:h + 1])
                                        diag[qt] = dg
                                ptb = tpool.tile([P, 2, S], bf16, tag="ptb")
                                pt_ps = psT.tile([P, 2 * S], f32, tag="ptps")
                                for kc in range(2):
                                    for qt in range(2):
                                        nc.tensor.matmul(
                                            pt_ps[:, kc * S + qt * P:
                                                  kc * S + (qt + 1) * P],
                                            lhsT=Ps[qt, fi][:, h % 2,
                                                            kc * P:(kc + 1) * P],
                                            rhs=(ident_bf[:]
                                                 if nmode.startswith("norm")
                                                 else diag[qt][:]),
                                            start=True, stop=True)
                                copy_on(ecfg["ptb"][h % 2], ptb[:], pt_ps)
                                if h % 4 == 0:
                                    avl = psV.tile([P, 2 * S], f32, tag="avps")
                                hh = h % 2
                                jj = (h % 4) // 2
                                for kc in range(2):
                                    nc.tensor.matmul(
                                        avl[hh * DH:(hh + 1) * DH,
                                            jj * S:(jj + 1) * S],
                                        lhsT=vU[:, 2 * bb + kc,
                                                h * DH:(h + 1) * DH],
                                        rhs=ptb[:, kc, :],
                                        start=(kc == 0), stop=(kc == 1))
                                if h % 4 == 3:
                                    j2 = h // 4  # f-tile pair index (0 or 1)
                                    dst = attnT[:, 2 * j2:2 * j2 + 2,
                                                boff:boff + S]
                                    copy_on(ecfg["avl"], dst, avl)
                        # --- O-projection + residual: psum = Wo^T attnT + z2
                        # (hbf already carries BN2(z_prev)).
                        for f in range(NF):
                            ps = psA.tile([P, CH], f32, tag="psA")
                            for kc in range(NF):
                                nc.tensor.matmul(
                                    ps, lhsT=w["wo"][:, kc, f * P:(f + 1) * P],
                                    rhs=attnT[:, kc, :],
                                    start=(kc == 0), stop=False)
                            nc.tensor.matmul(
                                ps, lhsT=ident_bf[:], rhs=hbf[:, f, :],
                                start=False, stop=True)
                            nc.scalar.copy(out=hT[:, f, tsl], in_=ps)
                            nc.vector.bn_stats(out=stats1[:, f, c, :],
                                               in_=hT[:, f, tsl])

                    a1, c1 = bn_allreduce(stats1, vec["g1"], vec["be1"],
                                          "bn1", sfx)

                    # ======================= phase B: FFN =====================
                    # BN1 is applied inside the bf16 cast: h1bf = BN1(z1);
                    # the residual is then a plain identity accumulate.
                    stats2 = stat.tile([P, NF, NCH, 6], f32, tag="st2")
                    for c in range(NCH):
                        tsl = slice(c * CH, (c + 1) * CH)
                        h1bf = cpool2.tile([P, NF, CH], bf16, tag="hbf",
                                           name=f"h1bf_{l}_{c}{sfx}")
                        for f in range(NF):
                            affine_cast_on(
                                ecfg["hbf"][1], h1bf[:, f, :], hT[:, f, tsl],
                                a1[:, f:f + 1], c1[:, f:f + 1])
                        ffn = cpool2.tile([P, NF, CH], bf16, tag="ffn")
                        for f in range(NF):
                            ps = psA.tile([P, CH], f32, tag="psA")
                            for kc in range(NF):
                                nc.tensor.matmul(
                                    ps, lhsT=w["w1"][:, kc, f * P:(f + 1) * P],
                                    rhs=h1bf[:, kc, :],
                                    start=(kc == 0), stop=(kc == NF - 1))
                            nc.scalar.activation(out=ffn[:, f, :], in_=ps,
                                                 func=Act.Relu,
                                                 bias=vec["b1"][:, f:f + 1])
                        # W2 + residual: psum = W2^T ffn + h1bf (= BN1(z1))
                        hdst = hTb if l == L - 1 else hT
                        for f in range(NF):
                            ps = psA.tile([P, CH], f32, tag="psA")
                            for kc in range(NF):
                                nc.tensor.matmul(
                                    ps, lhsT=w["w2"][:, kc, f * P:(f + 1) * P],
                                    rhs=ffn[:, kc, :],
                                    start=(kc == 0), stop=False)
                            nc.tensor.matmul(
                                ps, lhsT=ident_bf[:], rhs=h1bf[:, f, :],
                                start=False, stop=True)
                            nc.scalar.copy(out=hdst[:, f, tsl], in_=ps)
                            nc.vector.bn_stats(out=stats2[:, f, c, :],
                                               in_=hdst[:, f, tsl])

                    a_pend, c_pend = bn_allreduce(stats2, vec["g2"],
                                                  vec["be2"], "bn2", sfx)

                # ======== final: fused BN2-apply + transpose + store ========
                # out[t, d] = a2[d]*z[d, t] + c2[d], via bf16 matmuls with
                # diag(a2) plus a rank-1 (ones x c2row) accumulate.
                diagAF = build_diag(a_pend, f"final{sfx}")
                crow_ps = psS.tile([1, 2 * S], f32, tag="sps",
                                   name=f"crow_ps{sfx}")
                cpbf = stat.tile([P, NF], bf16, tag="cpbf",
                                 name=f"cpbf{sfx}")
                nc.vector.tensor_copy(out=cpbf[:], in_=c_pend[:])
                for f in range(NF):
                    nc.tensor.matmul(
                        crow_ps[0:1, f * P:(f + 1) * P],
                        lhsT=cpbf[:, f:f + 1], rhs=ident_bf[:],
                        start=True, stop=True)
                crow = const.tile([1, NF * P], bf16, tag="crow")
                nc.vector.tensor_copy(out=crow[:],
                                      in_=crow_ps[0:1, :NF * P])
                for tt in range(T // P):
                    ops = psA.tile([P, CH], f32, tag="psA")
                    for f in range(NF):
                        nc.tensor.matmul(
                            ops[:, f * P:(f + 1) * P],
                            lhsT=hTb[:, f, tt * P:(tt + 1) * P],
                            rhs=diagAF[:, f, :],
                            start=True, stop=False)
                        nc.tensor.matmul(
                            ops[:, f * P:(f + 1) * P],
                            lhsT=ones_bf[:], rhs=crow[0:1, f * P:(f + 1) * P],
                            start=False, stop=True)
                    ot = opool.tile([P, CH], f32, tag="ot")
                    if tt % 2 == 0:
                        nc.vector.tensor_copy(out=ot[:], in_=ops)
                    else:
                        nc.scalar.copy(out=ot[:], in_=ops)
                    nc.sync.dma_start(out_d[tt * P:(tt + 1) * P, :], ot[:])

    nc.compile()
    return nc


def _host_prep(inputs):
    """Build per-core in_maps from the full inputs."""
    seq = np.asarray(inputs["sequence"])
    pos = np.asarray(inputs["pos_encodings"], dtype=np.float32)
    emb = np.asarray(inputs["embed"], dtype=np.float32)

    # extended embedding table: rows 0..95 vocab, 96..351 positions, pad to 384
    embt = np.zeros((KV * P, D), np.float32)
    embt[:V] = emb
    embt[V:V + S] = pos
    embt = np.ascontiguousarray(
        embt.reshape(KV, P, D).transpose(1, 0, 2))          # [P, KV, D]

    def wprep(wa):  # [L, D, X] -> [L, P, NF, X] bf16
        wa = np.asarray(wa, dtype=np.float32)
        return np.ascontiguousarray(
            wa.reshape(L, NF, P, wa.shape[-1]).transpose(0, 2, 1, 3)
        ).astype(_BF16)

    def vprep(va):  # [L, D] -> [L, P, NF] f32
        va = np.asarray(va, dtype=np.float32)
        return np.ascontiguousarray(va.reshape(L, NF, P).transpose(0, 2, 1))

    shared = {
        "embt": embt.astype(_BF16),
        "wq": wprep(inputs["Wq"]), "wk": wprep(inputs["Wk"]),
        "wv": wprep(inputs["Wv"]), "wo": wprep(inputs["Wo"]),
        "w1": wprep(inputs["W1"]), "w2": wprep(inputs["W2"]),
        # scores are scaled by 1/sqrt(DH)=1/8 during the Q copy via
        # activation(scale=0.125), which computes in*scale + bias — so the
        # q bias must be pre-scaled here.
        "bq": vprep(np.asarray(inputs["bq"], np.float32) * 0.125),
        "b1": vprep(inputs["b1"]),
        "g1": vprep(inputs["gamma1"]), "be1": vprep(inputs["beta1"]),
        "g2": vprep(inputs["gamma2"]), "be2": vprep(inputs["beta2"]),
    }

    in_maps = []
    for core in range(N_CORES):
        sl = seq[core * BL:(core + 1) * BL].reshape(T).astype(np.int64)
        onehot = np.zeros((KV * P, T), np.float32)
        tok = np.arange(T)
        onehot[sl, tok] = 1.0                       # vocab row
        onehot[V + (tok % S), tok] = 1.0            # position row
        onehot = np.ascontiguousarray(
            onehot.reshape(KV, P, T).transpose(1, 0, 2)).astype(_BF16)
        m = dict(shared)
        m["onehot"] = onehot
        in_maps.append(m)
    return in_maps


def _run(inputs, trace=False):
    from concourse import bass_utils
    if "nc" not in _cache:
        _cache["nc"] = _build_bass()
    nc = _cache["nc"]
    in_maps = _host_prep(inputs)
    res = bass_utils.run_bass_kernel_spmd(
        nc, in_maps, core_ids=list(range(N_CORES)), trace=trace)
    outs = [r["out"].reshape(BL, S, D) for r in res.results]
    full = np.concatenate(outs, axis=0).astype(np.float32)
    return full, res


def kernel(**inputs) -> np.ndarray:
    full, _ = _run(inputs, trace=False)
    return full
